# revision 24
# baseline (speedup 1.0000x reference)
"""Trainium2 Bass kernel for nn_ConvBranch: strided-conv front end + 4 Mamba
layers + final LN + x4 upsample.

Sharding (8 cores): core c = (batch b = c//2, d_inner half j = c%2).
Each core: its batch, full sequence T=2048 (post-conv), full d_model=512,
its 512-channel half of d_inner=1024.  Contractions over d_inner (x_proj,
out_proj) produce partial sums -> pair AllReduce ([0,1],[2,3],[4,5],[6,7])
in bf16.

v2 layout/perf notes:
- B/C rows for the scan are broadcast to 128 partitions by stride-0 DMA
  reads from the AllReduce output in DRAM (no PE one-hot matmuls, no
  PSUM->SBUF copies).
- y = sum_n h_n*C_n accumulated on the PE via bf16 identity matmuls into
  PSUM (removes ~1000 DVE/GPSIMD adds).
- LN: stats from a bf16 copy of h; mean folded into in_proj as a rank-1
  correction (host-precomputed -colsum(W)), rstd broadcast via one K=1
  matmul.  All GEMMs bf16 (1 cyc/row).
- Per layer: phase A (LN+in_proj+dconv+x_proj+AllReduce for all chunks)
  then phase B (dt+scan+gate+out_proj+per-chunk y AllReduce) so collective
  latency overlaps compute.
"""

import sys

import numpy as np

sys.path.insert(0, "/opt/trn_rl_repo")

B_ = 4
D_IN = 256
D = 512          # d_model
STRIDE = 4
KF = 8           # front conv kernel
DS = 16          # d_state
DCONV = 4
DI = 1024        # d_inner
DH = DI // 2     # per-core d_inner half
R = 32           # dt_rank
LN_EPS = 1e-5
P = 128
G = DH // P      # 4
FT = D // P      # 4
XP = 64          # x_proj rows: [dt 0:32 | B 32:48 | C 48:64]
GROUPS = [[0, 1], [2, 3], [4, 5], [6, 7]]


# ====================================================================== build
def build_program(T=2048, NL=4, TC=512):
    import contextlib

    import concourse.bacc as bacc
    import concourse.bass as bass
    import concourse.mybir as mybir
    from concourse.tile import TileContext

    F32 = mybir.dt.float32
    BF16 = mybir.dt.bfloat16
    AF = mybir.ActivationFunctionType
    OP = mybir.AluOpType

    TC = min(TC, T)
    NC = T // TC
    assert TC <= 512
    T_IN = T * STRIDE

    nc = bacc.Bacc("TRN2", target_bir_lowering=False, debug=False,
                   enable_asserts=False, num_devices=8)

    xcatT = nc.dram_tensor("xcatT", [2 * STRIDE * D_IN, T], BF16, kind="ExternalInput")
    wconv = nc.dram_tensor("wconv", [2 * STRIDE * D_IN, D], BF16, kind="ExternalInput")
    conv_bias = nc.dram_tensor("conv_bias", [D, 1], F32, kind="ExternalInput")
    w_in = nc.dram_tensor("w_in", [NL, D, 2 * DH], BF16, kind="ExternalInput")
    b_in = nc.dram_tensor("b_in", [NL, 2 * DH, 1], F32, kind="ExternalInput")
    wsum_neg = nc.dram_tensor("wsum_neg", [NL, 1, 2 * DH], BF16, kind="ExternalInput")
    dconv_wt = nc.dram_tensor("dconv_wt", [NL, DH, DCONV], F32, kind="ExternalInput")
    dconv_bt = nc.dram_tensor("dconv_bt", [NL, DH, 1], F32, kind="ExternalInput")
    w_xp = nc.dram_tensor("w_xp", [NL, DH, XP], BF16, kind="ExternalInput")
    w_dt = nc.dram_tensor("w_dt", [NL, R, DH], BF16, kind="ExternalInput")
    b_dt = nc.dram_tensor("b_dt", [NL, DH, 1], F32, kind="ExternalInput")
    a_cols = nc.dram_tensor("a_cols", [NL, DH, DS], F32, kind="ExternalInput")
    d_par = nc.dram_tensor("d_par", [NL, DH, 1], F32, kind="ExternalInput")
    w_out = nc.dram_tensor("w_out", [NL, DH, D], BF16, kind="ExternalInput")
    fn_wb = nc.dram_tensor("fn_wb", [D, 2], F32, kind="ExternalInput")
    identb_bf = nc.dram_tensor("identb_bf", [P, P], BF16, kind="ExternalInput")
    y_out = nc.dram_tensor("y_out", [T_IN, D], BF16, kind="ExternalOutput")

    NCK = NL * NC
    cc_dbc_i = [nc.dram_tensor(f"cc_dbc_i{k}", [XP, T // 2], BF16) for k in range(2 * NL)]
    cc_dbc_o = [nc.dram_tensor(f"cc_dbc_o{k}", [XP, T // 2], BF16) for k in range(2 * NL)]
    cc_y_i = [nc.dram_tensor(f"cc_y_i{k}", [D, TC], BF16) for k in range(NCK)]
    cc_y_o = [nc.dram_tensor(f"cc_y_o{k}", [D, TC], BF16) for k in range(NCK)]

    def bcast_rows(dram_rows):
        """[R, W] DRAM rows -> stride-0 AP readable as [P, R, W]."""
        return bass.AP(tensor=dram_rows.tensor, offset=dram_rows.offset,
                       ap=[[0, P]] + [list(d) for d in dram_rows.ap])

    with TileContext(nc) as tc, contextlib.ExitStack() as ctx:
        persist = ctx.enter_context(tc.tile_pool(name="persist", bufs=1))
        wpool = ctx.enter_context(tc.tile_pool(name="wpool", bufs=1))
        big = ctx.enter_context(tc.tile_pool(name="big", bufs=1))
        scanp = ctx.enter_context(tc.tile_pool(name="scanp", bufs=2))
        bc = ctx.enter_context(tc.tile_pool(name="bc", bufs=1))
        small = ctx.enter_context(tc.tile_pool(name="small", bufs=2))

        ones_col_bf = persist.tile([P, 1], BF16)
        nc.vector.memset(ones_col_bf, 1.0 / D)
        ones_row_bf = persist.tile([1, P], BF16)
        nc.vector.memset(ones_row_bf, 1.0)
        identb = persist.tile([P, P], BF16)
        nc.sync.dma_start(identb, identb_bf[:, :])
        identf = persist.tile([P, P], F32)
        nc.scalar.copy(identf, identb)
        eps_t = persist.tile([P, 1], F32)
        nc.vector.memset(eps_t, LN_EPS)

        h = [persist.tile([P, T], F32, name=f"h{f}") for f in range(FT)]

        # ------------------------------------------------- front conv + GELU
        with tc.tile_pool(name="convp", bufs=1) as convp, \
             tc.tile_pool(name="convx", bufs=4) as convx, \
             tc.tile_pool(name="convps", bufs=4, space="PSUM") as convps:
            K16 = (2 * STRIDE * D_IN) // P
            cb = []
            for f in range(FT):
                cbf = convp.tile([P, 1], F32, name=f"cb{f}")
                nc.sync.dma_start(cbf, conv_bias[f * P:(f + 1) * P, :])
                cb.append(cbf)
            for c in range(T // TC):
                pts = [convps.tile([P, TC], F32, tag="mm", name="mm")
                       for _ in range(FT)]
                for k in range(K16):
                    wt = convx.tile([P, D], BF16, tag="wc", name="wc", bufs=3)
                    nc.sync.dma_start(wt, wconv[k * P:(k + 1) * P, :])
                    xt = convx.tile([P, TC], BF16, tag="xcat", name="xcat")
                    nc.sync.dma_start(xt, xcatT[k * P:(k + 1) * P,
                                                c * TC:(c + 1) * TC])
                    for f in range(FT):
                        nc.tensor.matmul(pts[f], wt[:, f * P:(f + 1) * P],
                                         xt, start=(k == 0), stop=(k == K16 - 1))
                for f in range(FT):
                    nc.scalar.activation(h[f][:, c * TC:(c + 1) * TC], pts[f],
                                         AF.Gelu, bias=cb[f], scale=1.0)

        pa = ctx.enter_context(tc.tile_pool(name="pa", bufs=2, space="PSUM"))
        pb = ctx.enter_context(tc.tile_pool(name="pb", bufs=2, space="PSUM"))
        pyac = ctx.enter_context(tc.tile_pool(name="pyac", bufs=1, space="PSUM"))

        def ln_stats(sl, ci=0):
            """bf16 copy of h chunk + mean/rstd rows; returns (hc, s1, rstd)."""
            stat = pa.tile([P, TC], F32, tag="mm", name="stat")
            hc = []
            for f in range(FT):
                c = big.tile([P, TC], BF16, tag=f"hc{f}_{ci}", name=f"hc{f}")
                nc.scalar.copy(c, h[f][:, sl])
                hc.append(c)
                nc.tensor.matmul(stat[0:1, :], ones_col_bf, c,
                                 start=(f == 0), stop=(f == FT - 1))
            for f in range(FT):
                q = big.tile([P, TC], BF16, tag="hsq", name="hsq", bufs=1)
                nc.scalar.activation(q, hc[f], AF.Square)
                nc.tensor.matmul(stat[32:33, :], ones_col_bf, q,
                                 start=(f == 0), stop=(f == FT - 1))
            s1 = small.tile([1, TC], F32, tag="s1", name="s1", bufs=1)
            nc.scalar.copy(s1, stat[0:1, :])               # mean (ones = 1/D)
            msq = small.tile([1, TC], F32, tag="msq", name="msq", bufs=1)
            nc.scalar.activation(msq, s1, AF.Square)
            s2 = small.tile([1, TC], F32, tag="s2", name="s2", bufs=1)
            nc.vector.tensor_tensor(s2, stat[32:33, :], msq, op=OP.subtract)
            nc.scalar.activation(s2, s2, AF.Ln, bias=eps_t[0:1, :], scale=1.0)
            nc.scalar.activation(s2, s2, AF.Exp, scale=-0.5)  # rstd
            return hc, s1, s2

        def rep_row(row_bf):
            """Broadcast a [1, TC] bf16 row to a [P, TC] bf16 tile via PE."""
            rp = pa.tile([P, TC], F32, tag="mm", name="rep")
            nc.tensor.matmul(rp, ones_row_bf, row_bf, start=True, stop=True)
            out = big.tile([P, TC], BF16, tag="rrep", name="rrep", bufs=4)
            nc.scalar.copy(out, rp)
            return out

        # ---------------------------------------------------------- layers
        for l in range(NL):
            w_in_t = [wpool.tile([P, 2 * DH], BF16, tag=f"w_in{k}",
                                 name=f"w_in{k}") for k in range(FT)]
            for k in range(FT):
                nc.sync.dma_start(w_in_t[k], w_in[l, k * P:(k + 1) * P, :])
            wsum_t = wpool.tile([1, 2 * DH], BF16, tag="wsum", name="wsum")
            nc.sync.dma_start(wsum_t, wsum_neg[l])
            b_in_t = [wpool.tile([P, 1], F32, tag=f"b_in{e}", name=f"b_in{e}")
                      for e in range(2 * DH // P)]
            for e in range(2 * DH // P):
                nc.sync.dma_start(b_in_t[e], b_in[l, e * P:(e + 1) * P, :])
            dcw_t = [wpool.tile([P, DCONV], F32, tag=f"dcw{g}", name=f"dcw{g}")
                     for g in range(G)]
            dcb_t = [wpool.tile([P, 1], F32, tag=f"dcb{g}", name=f"dcb{g}")
                     for g in range(G)]
            w_xp_t = [wpool.tile([P, XP], BF16, tag=f"w_xp{g}", name=f"w_xp{g}")
                      for g in range(G)]
            b_dt_t = [wpool.tile([P, 1], F32, tag=f"b_dt{g}", name=f"b_dt{g}")
                      for g in range(G)]
            ac_t = [wpool.tile([P, DS], F32, tag=f"ac{g}", name=f"ac{g}")
                    for g in range(G)]
            dpar_t = [wpool.tile([P, 1], F32, tag=f"dpar{g}", name=f"dpar{g}")
                      for g in range(G)]
            w_out_t = [wpool.tile([P, D], BF16, tag=f"w_out{g}", name=f"w_out{g}")
                       for g in range(G)]
            for g in range(G):
                s = slice(g * P, (g + 1) * P)
                nc.sync.dma_start(dcw_t[g], dconv_wt[l, s, :])
                nc.sync.dma_start(dcb_t[g], dconv_bt[l, s, :])
                nc.sync.dma_start(w_xp_t[g], w_xp[l, s, :])
                nc.sync.dma_start(b_dt_t[g], b_dt[l, s, :])
                nc.sync.dma_start(ac_t[g], a_cols[l, s, :])
                nc.sync.dma_start(dpar_t[g], d_par[l, s, :])
                nc.sync.dma_start(w_out_t[g], w_out[l, s, :])
            w_dt_t = wpool.tile([R, DH], BF16, tag="w_dt", name="w_dt")
            nc.sync.dma_start(w_dt_t, w_dt[l])

            # full-T concat buffers (xs written in-place over xraw after dconv)
            # pad to 4 so the xs view starts at an even element offset (DVE 2x)
            PAD = 4
            xraw_c = [big.tile([P, PAD + T], BF16, tag=f"xrc{g}",
                               name=f"xrc{g}") for g in range(G)]
            zs_c = [big.tile([P, T], BF16, tag=f"zsc{g}", name=f"zsc{g}")
                    for g in range(G)]
            for g in range(G):
                nc.vector.memset(xraw_c[g][:, 0:PAD], 0.0)

            # ===== phase A: stats for all chunks first (cross-chunk pipelining)
            srows = []
            for ci in range(NC):
                t0 = ci * TC
                hc, s1, s2 = ln_stats(slice(t0, t0 + TC), ci)
                rstd_bf = small.tile([1, TC], BF16, tag="rb", name="rb", bufs=1)
                nc.scalar.copy(rstd_bf, s2)
                mr_row = small.tile([1, TC], BF16, tag=f"mr{ci}", name="mr",
                                    bufs=1)
                nc.vector.tensor_tensor(mr_row, s1, s2, op=OP.mult)
                srows.append((hc, mr_row, rep_row(rstd_bf)))
            # ===== in_proj per chunk into concat buffers
            for ci in range(NC):
                t0 = ci * TC
                sl = slice(t0, t0 + TC)
                hc, mr_row, rrep = srows[ci]
                hcn = hc
                for f in range(FT):
                    nc.vector.tensor_tensor(hcn[f], hc[f], rrep, op=OP.mult)
                for half in range(2):
                    for g in range(G):
                        e0 = half * DH + g * P
                        pt = pa.tile([P, TC], F32, tag="mm", name="mm")
                        for k in range(FT):
                            nc.tensor.matmul(pt, w_in_t[k][:, e0:e0 + P],
                                             hcn[k], start=(k == 0), stop=False)
                        nc.tensor.matmul(pt, wsum_t[:, e0:e0 + P], mr_row,
                                         start=False, stop=True)
                        bia = b_in_t[e0 // P]
                        if half == 0:
                            nc.scalar.activation(
                                xraw_c[g][:, PAD + t0:PAD + t0 + TC],
                                pt, AF.Identity, bias=bia, scale=1.0)
                        else:
                            nc.scalar.activation(zs_c[g][:, sl], pt, AF.Silu,
                                                 bias=bia, scale=1.0)

            # ============ dconv on gpsimd, per T-half (in-place silu -> xs)
            TH = T // 2
            xs_c = []
            for g in range(G):
                for hh in range(2):
                    o0 = hh * TH
                    tmp = small.tile([P, TH], BF16, tag="dctmp", name="dctmp",
                                     bufs=2)
                    nc.vector.tensor_scalar(tmp, xraw_c[g][:, 1 + o0:1 + o0 + TH],
                                            dcw_t[g][:, 0:1], None, OP.mult)
                    for j in range(1, DCONV):
                        tmp2 = small.tile([P, TH], BF16, tag="dctmp",
                                          name="dctmp", bufs=2)
                        nc.vector.scalar_tensor_tensor(
                            tmp2, xraw_c[g][:, 1 + j + o0:1 + j + o0 + TH],
                            dcw_t[g][:, j:j + 1], tmp, op0=OP.mult, op1=OP.add)
                        tmp = tmp2
                    nc.scalar.activation(xraw_c[g][:, PAD + o0:PAD + o0 + TH],
                                         tmp, AF.Silu, bias=dcb_t[g],
                                         scale=1.0)
                xs_c.append(xraw_c[g][:, PAD:PAD + T])

            # ===== x_proj partials + pair AllReduce per T-half (bf16)
            T2 = T // 2
            for hf in range(2):
                for cj in range(2):
                    t0 = (2 * hf + cj) * TC
                    pt = pa.tile([P, TC], F32, tag="mm", name="mm")
                    for g in range(G):
                        nc.tensor.matmul(pt[0:XP, :], w_xp_t[g],
                                         xs_c[g][:, t0:t0 + TC],
                                         start=(g == 0), stop=(g == G - 1))
                    dbc_p = small.tile([XP, TC], BF16, tag="dbc_p",
                                       name="dbc_p", bufs=1)
                    nc.scalar.copy(dbc_p, pt[0:XP, :])
                    nc.sync.dma_start(
                        cc_dbc_i[2 * l + hf][:, cj * TC:(cj + 1) * TC], dbc_p)
                nc.gpsimd.collective_compute(
                    "AllReduce", OP.add, replica_groups=GROUPS,
                    ins=[cc_dbc_i[2 * l + hf][:, :]],
                    outs=[cc_dbc_o[2 * l + hf][:, :]])

            dt_c = [big.tile([P, T], BF16, tag=f"dtc{g}", name=f"dtc{g}")
                    for g in range(G)]
            dtu_c = [big.tile([P, T], BF16, tag=f"dtuc{g}", name=f"dtuc{g}")
                     for g in range(G)]
            fin = [big.tile([P, DS], F32, tag=f"fin{g}", name=f"fin{g}")
                   for g in range(G)]

            for hf in range(2):
                h0 = hf * T2
                hsl = slice(h0, h0 + T2)
                cc = cc_dbc_o[2 * l + hf]
                # ---- dt = softplus(w_dt @ dbc_dt + b) for this half
                dbc_dt = small.tile([R, T2], BF16, tag="dbc_dt", name="dbc_dt",
                                    bufs=1)
                nc.sync.dma_start(dbc_dt, cc[0:R, :])
                # softplus: all exps first, then all lns in-place (1 table swap)
                for g in range(G):
                    for cj in range(2):
                        ptd = pb.tile([P, TC], F32, tag="mm", name="mm")
                        nc.tensor.matmul(ptd, w_dt_t[:, g * P:(g + 1) * P],
                                         dbc_dt[:, cj * TC:(cj + 1) * TC],
                                         start=True, stop=True)
                        nc.scalar.activation(
                            dt_c[g][:, h0 + cj * TC:h0 + (cj + 1) * TC], ptd,
                            AF.Exp, bias=b_dt_t[g], scale=1.0)
                for g in range(G):
                    for cj in range(2):
                        sl2 = slice(h0 + cj * TC, h0 + (cj + 1) * TC)
                        nc.scalar.activation(dt_c[g][:, sl2], dt_c[g][:, sl2],
                                             AF.Ln, bias=1.0, scale=1.0)
                for g in range(G):
                    nc.vector.tensor_tensor(dtu_c[g][:, hsl], dt_c[g][:, hsl],
                                            xs_c[g][:, hsl], op=OP.mult)

                # ---- scan blocks: g-pairs share B/C fetches
                for gp in ((0, 1), (2, 3)):
                    ypgs = {}
                    for g in gp:
                        yp = pyac.tile([P, T2], F32, tag=f"y{g % 2}",
                                       name=f"y{g % 2}")
                        for cj in range(2):
                            xsD = small.tile([P, TC], BF16, tag="xsD",
                                             name="xsD", bufs=1)
                            nc.vector.tensor_scalar(
                                xsD, xs_c[g][:, h0 + cj * TC:h0 + (cj + 1) * TC],
                                dpar_t[g], None, OP.mult)
                            nc.tensor.matmul(yp[:, cj * TC:(cj + 1) * TC],
                                             identb, xsD, start=True,
                                             stop=False)
                        ypgs[g] = yp

                    def fetch_q(q):
                        """One DMA per 2-row pair of B rows / C rows."""
                        bt = bc.tile([P, 2 * T2], BF16, tag=f"bq{q % 2}",
                                     name=f"bq{q % 2}")
                        ct = bc.tile([P, 2 * T2], BF16, tag=f"cq{q % 2}",
                                     name=f"cq{q % 2}")
                        bdst = bass.AP(tensor=bt.tensor, offset=bt.offset,
                                       ap=[list(bt.ap[0]), [T2, 2], [1, T2]])
                        cdst = bass.AP(tensor=ct.tensor, offset=ct.offset,
                                       ap=[list(ct.ap[0]), [T2, 2], [1, T2]])
                        nc.scalar.dma_start(
                            bdst, bcast_rows(cc[R + 2 * q:R + 2 * q + 2, :]),
                            single_packet=True)
                        nc.gpsimd.dma_start(
                            cdst,
                            bcast_rows(cc[R + DS + 2 * q:R + DS + 2 * q + 2, :]),
                            single_packet=True)
                        return bt, ct

                    bcache = {0: fetch_q(0), 1: fetch_q(1)}
                    for q in range(8):
                        bt_q, ct_q = bcache[q % 2]
                        for ni in range(2):
                            if ni == 1 and 0 < q < 7:
                                # prefetch q+1 here so the DMA transfer
                                # overlaps ni=1's scans, not the cm/b_t ops
                                bcache[(q + 1) % 2] = fetch_q(q + 1)
                            n = 2 * q + ni
                            brep = bt_q[:, ni * T2:(ni + 1) * T2]
                            crep = ct_q[:, ni * T2:(ni + 1) * T2]
                            hscs = {}
                            for g in gp:
                                a_t = scanp.tile([P, T2], BF16, tag="sa",
                                                 name="sa", bufs=2)
                                nc.scalar.activation(a_t, dt_c[g][:, hsl],
                                                     AF.Exp,
                                                     scale=ac_t[g][:, n:n + 1])
                                b_t = scanp.tile([P, T2], BF16, tag="sb",
                                                 name="sb")
                                nc.vector.tensor_tensor(b_t, dtu_c[g][:, hsl],
                                                        brep, op=OP.mult)
                                hsc = scanp.tile([P, T2], BF16, tag="sh",
                                                 name="sh")
                                init = 0.0 if hf == 0 else fin[g][:, n:n + 1]
                                nc.vector.tensor_tensor_scan(hsc, a_t, b_t,
                                                             init,
                                                             op0=OP.mult,
                                                             op1=OP.add)
                                if hf == 0:
                                    nc.vector.tensor_copy(fin[g][:, n:n + 1],
                                                          hsc[:, T2 - 1:T2])
                                hscs[g] = hsc
                            # cm after both scans (avoid read-after-write
                            # stall on hsc); one per engine per n
                            for gi, g in enumerate(gp):
                                cm = scanp.tile([P, T2], BF16, tag="sm",
                                                name="sm")
                                eng = nc.gpsimd if ((n + gi) % 2 == 1) \
                                    else nc.vector
                                eng.tensor_tensor(cm, hscs[g], crep,
                                                  op=OP.mult)
                                for cj in range(2):
                                    nc.tensor.matmul(
                                        ypgs[g][:, cj * TC:(cj + 1) * TC],
                                        identb, cm[:, cj * TC:(cj + 1) * TC],
                                        start=False, stop=(n == DS - 1))
                    # gating into dtu_c (dead after n loop)
                    for g in gp:
                        nc.vector.tensor_tensor(dtu_c[g][:, hsl], ypgs[g],
                                                zs_c[g][:, hsl], op=OP.mult)

                # ---- out_proj + AR_y + residual for this half's chunks
                for cj in range(2):
                    ci = 2 * hf + cj
                    t0 = ci * TC
                    sl = slice(t0, t0 + TC)
                    ccidx = l * NC + ci
                    for f in range(FT):
                        po = pb.tile([P, TC], F32, tag="mm", name="mm")
                        for g in range(G):
                            nc.tensor.matmul(po,
                                             w_out_t[g][:, f * P:(f + 1) * P],
                                             dtu_c[g][:, sl], start=(g == 0),
                                             stop=(g == G - 1))
                        ot = small.tile([P, TC], BF16, tag="oout", name="oout")
                        nc.scalar.copy(ot, po)
                        nc.sync.dma_start(cc_y_i[ccidx][f * P:(f + 1) * P, :],
                                          ot)
                    nc.gpsimd.collective_compute(
                        "AllReduce", OP.add, replica_groups=GROUPS,
                        ins=[cc_y_i[ccidx][:, :]], outs=[cc_y_o[ccidx][:, :]])
                    for f in range(FT):
                        yfull = scanp.tile([P, TC], BF16, tag="yfull",
                                           name="yfull", bufs=2)
                        nc.sync.dma_start(yfull,
                                          cc_y_o[ccidx][f * P:(f + 1) * P, :])
                        nc.gpsimd.tensor_tensor(h[f][:, sl], h[f][:, sl],
                                                yfull, op=OP.add)

        # ------------------------------------------- final LN + transpose out
        fn_t = [persist.tile([P, 2], F32, name=f"fn{f}") for f in range(FT)]
        for f in range(FT):
            nc.sync.dma_start(fn_t[f], fn_wb[f * P:(f + 1) * P, :])
        for ci in range(NC):
            t0 = ci * TC
            sl = slice(t0, t0 + TC)
            hc, s1, s2 = ln_stats(sl)
            s1_bf = small.tile([1, TC], BF16, tag="mr", name="mr", bufs=1)
            nc.vector.tensor_copy(s1_bf, s1)
            rstd_bf = small.tile([1, TC], BF16, tag="rb", name="rb", bufs=1)
            nc.vector.tensor_copy(rstd_bf, s2)
            mrep = rep_row(s1_bf)
            rrep = rep_row(rstd_bf)
            hn = []
            for f in range(FT):
                t2 = big.tile([P, TC], F32, tag=f"fhn{f}", name=f"hn{f}")
                nc.vector.tensor_tensor(t2, hc[f], mrep, op=OP.subtract)
                nc.vector.tensor_tensor(t2, t2, rrep, op=OP.mult)
                nc.vector.tensor_scalar(t2, t2, fn_t[f][:, 0:1],
                                        fn_t[f][:, 1:2], OP.mult, OP.add)
                hn.append(t2)
            for tb in range(TC // P):
                ht = small.tile([P, D], BF16, tag="ht", name="ht", bufs=1)
                for f in range(FT):
                    pt = pa.tile([P, TC], F32, tag="mm", name="mm")
                    nc.tensor.transpose(pt[:, 0:P], hn[f][:, tb * P:(tb + 1) * P],
                                        identf)
                    nc.scalar.copy(ht[:, f * P:(f + 1) * P], pt[:, 0:P])
                tglob = t0 + tb * P
                src = ht[:, :]
                rep_in = bass.AP(tensor=src.tensor, offset=src.offset,
                                 ap=[list(src.ap[0]), [0, STRIDE],
                                     list(src.ap[1])])
                dst = y_out[STRIDE * tglob:STRIDE * (tglob + P), :]
                dst3 = dst.rearrange("(t r) d -> t r d", r=STRIDE)
                nc.sync.dma_start(dst3, rep_in)

    nc.compile()
    return nc


# ================================================================ host side
def make_core_inputs(inputs, T=2048, NL=4):
    x = np.asarray(inputs["x"], np.float32)
    conv_w = np.asarray(inputs["conv_w"], np.float32)
    conv_b = np.asarray(inputs["conv_b"], np.float32)
    in_proj_w = np.asarray(inputs["in_proj_w"], np.float32)
    dconv_w = np.asarray(inputs["dconv_w"], np.float32)
    dconv_b = np.asarray(inputs["dconv_b"], np.float32)
    x_proj_w = np.asarray(inputs["x_proj_w"], np.float32)
    dt_proj_w = np.asarray(inputs["dt_proj_w"], np.float32)
    dt_proj_b = np.asarray(inputs["dt_proj_b"], np.float32)
    A_log = np.asarray(inputs["A_log"], np.float32)
    D_param = np.asarray(inputs["D_param"], np.float32)
    out_proj_w = np.asarray(inputs["out_proj_w"], np.float32)
    ln_w = np.asarray(inputs["ln_w"], np.float32)
    ln_b = np.asarray(inputs["ln_b"], np.float32)
    fn_w = np.asarray(inputs["fn_w"], np.float32)
    fn_b = np.asarray(inputs["fn_b"], np.float32)

    Bn = x.shape[0]
    di = x.shape[2]
    dmodel = conv_w.shape[0]
    dinner = in_proj_w.shape[1] // 2
    dh = dinner // 2

    xpad = np.concatenate([np.zeros((Bn, KF - 1, di), np.float32), x], axis=1)
    idx = np.arange(T)[:, None] * STRIDE + np.arange(KF)[None, :]
    xcat = xpad[:, idx, :].reshape(Bn, T, KF * di)
    xcatT = np.ascontiguousarray(xcat.transpose(0, 2, 1))
    wconv = np.ascontiguousarray(conv_w.transpose(2, 1, 0).reshape(KF * di, dmodel))

    A = -np.exp(A_log)

    per_core = []
    for c in range(8):
        b, j = c // 2, c % 2
        sl = slice(j * dh, (j + 1) * dh)
        w_in_l, b_in_l, w_out_l, w_xp_l, wsum_l = [], [], [], [], []
        for l in range(NL):
            Wx = in_proj_w[l, :dinner][sl] * ln_w[l][None, :]
            Wz = in_proj_w[l, dinner:][sl] * ln_w[l][None, :]
            wl = np.concatenate([Wx.T, Wz.T], axis=1)
            w_in_l.append(wl)
            wsum_l.append(-wl.sum(axis=0, keepdims=True))
            bx = in_proj_w[l, :dinner][sl] @ ln_b[l]
            bz = in_proj_w[l, dinner:][sl] @ ln_b[l]
            b_in_l.append(np.concatenate([bx, bz])[:, None])
            w_out_l.append(out_proj_w[l][:, sl].T)
            w_xp_l.append(np.ascontiguousarray(x_proj_w[l][:, sl].T))
        d = dict(
            xcatT=xcatT[b],
            wconv=wconv,
            conv_bias=conv_b[:, None],
            w_in=np.stack(w_in_l),
            b_in=np.stack(b_in_l),
            wsum_neg=np.stack(wsum_l),
            dconv_wt=dconv_w[:, sl, :],
            dconv_bt=dconv_b[:, sl, None],
            w_xp=np.stack(w_xp_l),
            w_dt=np.ascontiguousarray(dt_proj_w[:, sl, :].transpose(0, 2, 1)),
            b_dt=dt_proj_b[:, sl, None],
            a_cols=A[:, sl, :],
            d_par=D_param[:, sl, None],
            w_out=np.stack(w_out_l),
            fn_wb=np.stack([fn_w, fn_b], axis=1),
            identb_bf=np.eye(P, dtype=np.float32),
        )
        per_core.append(d)
    return per_core


def cast_core_inputs(nc, per_core):
    import concourse.mybir as mybir
    want = {}
    for alloc in nc.m.functions[0].allocations:
        if getattr(alloc, "kind", None) == "ExternalInput":
            want[alloc.memorylocations[0].name] = mybir.dt.np(alloc.dtype)
    return [{k: np.ascontiguousarray(np.asarray(v).astype(want[k]))
             for k, v in d.items() if k in want} for d in per_core]


_PROGRAM_CACHE = {}


def get_program(T=2048, NL=4, TC=512):
    key = (T, NL, TC)
    if key not in _PROGRAM_CACHE:
        _PROGRAM_CACHE[key] = build_program(T, NL, TC)
    return _PROGRAM_CACHE[key]


def kernel(**inputs):
    from concourse.bass_utils import run_bass_kernel_spmd
    T = inputs["x"].shape[1] // STRIDE
    NL = inputs["in_proj_w"].shape[0]
    nc = get_program(T, NL)
    per_core = cast_core_inputs(nc, make_core_inputs(inputs, T, NL))
    res = run_bass_kernel_spmd(nc, per_core, core_ids=list(range(8)))
    Bn = inputs["x"].shape[0]
    y = np.stack([res.results[2 * b]["y_out"] for b in range(Bn)])
    return y.astype(np.float32)



# revision 27
# speedup vs baseline: 1.0029x; 1.0029x over previous
"""Trainium2 Bass kernel for nn_ConvBranch: strided-conv front end + 4 Mamba
layers + final LN + x4 upsample.

Sharding (8 cores): core c = (batch b = c//2, d_inner half j = c%2).
Each core: its batch, full sequence T=2048 (post-conv), full d_model=512,
its 512-channel half of d_inner=1024.  Contractions over d_inner (x_proj,
out_proj) produce partial sums -> pair AllReduce ([0,1],[2,3],[4,5],[6,7])
in bf16.

v2 layout/perf notes:
- B/C rows for the scan are broadcast to 128 partitions by stride-0 DMA
  reads from the AllReduce output in DRAM (no PE one-hot matmuls, no
  PSUM->SBUF copies).
- y = sum_n h_n*C_n accumulated on the PE via bf16 identity matmuls into
  PSUM (removes ~1000 DVE/GPSIMD adds).
- LN: stats from a bf16 copy of h; mean folded into in_proj as a rank-1
  correction (host-precomputed -colsum(W)), rstd broadcast via one K=1
  matmul.  All GEMMs bf16 (1 cyc/row).
- Per layer: phase A (LN+in_proj+dconv+x_proj+AllReduce for all chunks)
  then phase B (dt+scan+gate+out_proj+per-chunk y AllReduce) so collective
  latency overlaps compute.
"""

import sys

import numpy as np

sys.path.insert(0, "/opt/trn_rl_repo")

B_ = 4
D_IN = 256
D = 512          # d_model
STRIDE = 4
KF = 8           # front conv kernel
DS = 16          # d_state
DCONV = 4
DI = 1024        # d_inner
DH = DI // 2     # per-core d_inner half
R = 32           # dt_rank
LN_EPS = 1e-5
P = 128
G = DH // P      # 4
FT = D // P      # 4
XP = 64          # x_proj rows: [dt 0:32 | B 32:48 | C 48:64]
GROUPS = [[0, 1], [2, 3], [4, 5], [6, 7]]


# ====================================================================== build
def build_program(T=2048, NL=4, TC=512):
    import contextlib

    import concourse.bacc as bacc
    import concourse.bass as bass
    import concourse.mybir as mybir
    from concourse.tile import TileContext

    F32 = mybir.dt.float32
    BF16 = mybir.dt.bfloat16
    AF = mybir.ActivationFunctionType
    OP = mybir.AluOpType

    TC = min(TC, T)
    NC = T // TC
    assert TC <= 512
    T_IN = T * STRIDE

    nc = bacc.Bacc("TRN2", target_bir_lowering=False, debug=False,
                   enable_asserts=False, num_devices=8)

    xcatT = nc.dram_tensor("xcatT", [2 * STRIDE * D_IN, T], BF16, kind="ExternalInput")
    wconv = nc.dram_tensor("wconv", [2 * STRIDE * D_IN, D], BF16, kind="ExternalInput")
    conv_bias = nc.dram_tensor("conv_bias", [D, 1], F32, kind="ExternalInput")
    w_in = nc.dram_tensor("w_in", [NL, D, 2 * DH], BF16, kind="ExternalInput")
    b_in = nc.dram_tensor("b_in", [NL, 2 * DH, 1], F32, kind="ExternalInput")
    wsum_neg = nc.dram_tensor("wsum_neg", [NL, 1, 2 * DH], BF16, kind="ExternalInput")
    dconv_wt = nc.dram_tensor("dconv_wt", [NL, DH, DCONV], F32, kind="ExternalInput")
    dconv_bt = nc.dram_tensor("dconv_bt", [NL, DH, 1], F32, kind="ExternalInput")
    w_xp = nc.dram_tensor("w_xp", [NL, DH, XP], BF16, kind="ExternalInput")
    w_dt = nc.dram_tensor("w_dt", [NL, R, DH], BF16, kind="ExternalInput")
    b_dt = nc.dram_tensor("b_dt", [NL, DH, 1], F32, kind="ExternalInput")
    a_cols = nc.dram_tensor("a_cols", [NL, DH, DS], F32, kind="ExternalInput")
    d_par = nc.dram_tensor("d_par", [NL, DH, 1], F32, kind="ExternalInput")
    w_out = nc.dram_tensor("w_out", [NL, DH, D], BF16, kind="ExternalInput")
    fn_wb = nc.dram_tensor("fn_wb", [D, 2], F32, kind="ExternalInput")
    identb_bf = nc.dram_tensor("identb_bf", [P, P], BF16, kind="ExternalInput")
    y_out = nc.dram_tensor("y_out", [T_IN, D], BF16, kind="ExternalOutput")

    NCK = NL * NC
    cc_dbc_i = [nc.dram_tensor(f"cc_dbc_i{k}", [XP, T // 2], BF16) for k in range(2 * NL)]
    cc_dbc_o = [nc.dram_tensor(f"cc_dbc_o{k}", [XP, T // 2], BF16) for k in range(2 * NL)]
    cc_y_i = [nc.dram_tensor(f"cc_y_i{k}", [D, TC], BF16) for k in range(NCK)]
    cc_y_o = [nc.dram_tensor(f"cc_y_o{k}", [D, TC], BF16) for k in range(NCK)]

    def bcast_rows(dram_rows):
        """[R, W] DRAM rows -> stride-0 AP readable as [P, R, W]."""
        return bass.AP(tensor=dram_rows.tensor, offset=dram_rows.offset,
                       ap=[[0, P]] + [list(d) for d in dram_rows.ap])

    with TileContext(nc) as tc, contextlib.ExitStack() as ctx:
        persist = ctx.enter_context(tc.tile_pool(name="persist", bufs=1))
        wpool = ctx.enter_context(tc.tile_pool(name="wpool", bufs=1))
        big = ctx.enter_context(tc.tile_pool(name="big", bufs=1))
        scanp = ctx.enter_context(tc.tile_pool(name="scanp", bufs=2))
        bc = ctx.enter_context(tc.tile_pool(name="bc", bufs=1))
        small = ctx.enter_context(tc.tile_pool(name="small", bufs=2))

        ones_col_bf = persist.tile([P, 1], BF16)
        nc.vector.memset(ones_col_bf, 1.0 / D)
        ones_row_bf = persist.tile([1, P], BF16)
        nc.vector.memset(ones_row_bf, 1.0)
        identb = persist.tile([P, P], BF16)
        nc.sync.dma_start(identb, identb_bf[:, :])
        identf = persist.tile([P, P], F32)
        nc.scalar.copy(identf, identb)
        eps_t = persist.tile([P, 1], F32)
        nc.vector.memset(eps_t, LN_EPS)

        h = [persist.tile([P, T], F32, name=f"h{f}") for f in range(FT)]

        # ------------------------------------------------- front conv + GELU
        with tc.tile_pool(name="convp", bufs=1) as convp, \
             tc.tile_pool(name="convx", bufs=4) as convx, \
             tc.tile_pool(name="convps", bufs=4, space="PSUM") as convps:
            K16 = (2 * STRIDE * D_IN) // P
            cb = []
            for f in range(FT):
                cbf = convp.tile([P, 1], F32, name=f"cb{f}")
                nc.sync.dma_start(cbf, conv_bias[f * P:(f + 1) * P, :])
                cb.append(cbf)
            for c in range(T // TC):
                pts = [convps.tile([P, TC], F32, tag="mm", name="mm")
                       for _ in range(FT)]
                for k in range(K16):
                    wt = convx.tile([P, D], BF16, tag="wc", name="wc", bufs=2)
                    nc.sync.dma_start(wt, wconv[k * P:(k + 1) * P, :])
                    xt = convx.tile([P, TC], BF16, tag="xcat", name="xcat", bufs=2)
                    nc.sync.dma_start(xt, xcatT[k * P:(k + 1) * P,
                                                c * TC:(c + 1) * TC])
                    for f in range(FT):
                        nc.tensor.matmul(pts[f], wt[:, f * P:(f + 1) * P],
                                         xt, start=(k == 0), stop=(k == K16 - 1))
                for f in range(FT):
                    nc.scalar.activation(h[f][:, c * TC:(c + 1) * TC], pts[f],
                                         AF.Gelu, bias=cb[f], scale=1.0)

        pa = ctx.enter_context(tc.tile_pool(name="pa", bufs=2, space="PSUM"))
        pb = ctx.enter_context(tc.tile_pool(name="pb", bufs=2, space="PSUM"))
        pyac = ctx.enter_context(tc.tile_pool(name="pyac", bufs=1, space="PSUM"))

        def ln_stats(sl, ci=0):
            """bf16 copy of h chunk + mean/rstd rows; returns (hc, s1, rstd)."""
            stat = pa.tile([P, TC], F32, tag="mm", name="stat")
            hc = []
            for f in range(FT):
                c = big.tile([P, TC], BF16, tag=f"hc{f}_{ci}", name=f"hc{f}")
                nc.scalar.copy(c, h[f][:, sl])
                hc.append(c)
                nc.tensor.matmul(stat[0:1, :], ones_col_bf, c,
                                 start=(f == 0), stop=(f == FT - 1))
            for f in range(FT):
                q = big.tile([P, TC], BF16, tag="hsq", name="hsq", bufs=1)
                nc.scalar.activation(q, hc[f], AF.Square)
                nc.tensor.matmul(stat[32:33, :], ones_col_bf, q,
                                 start=(f == 0), stop=(f == FT - 1))
            s1 = small.tile([1, TC], F32, tag="s1", name="s1", bufs=1)
            nc.scalar.copy(s1, stat[0:1, :])               # mean (ones = 1/D)
            msq = small.tile([1, TC], F32, tag="msq", name="msq", bufs=1)
            nc.scalar.activation(msq, s1, AF.Square)
            s2 = small.tile([1, TC], F32, tag="s2", name="s2", bufs=1)
            nc.vector.tensor_tensor(s2, stat[32:33, :], msq, op=OP.subtract)
            nc.scalar.activation(s2, s2, AF.Ln, bias=eps_t[0:1, :], scale=1.0)
            nc.scalar.activation(s2, s2, AF.Exp, scale=-0.5)  # rstd
            return hc, s1, s2

        def rep_row(row_bf):
            """Broadcast a [1, TC] bf16 row to a [P, TC] bf16 tile via PE."""
            rp = pa.tile([P, TC], F32, tag="mm", name="rep")
            nc.tensor.matmul(rp, ones_row_bf, row_bf, start=True, stop=True)
            out = big.tile([P, TC], BF16, tag="rrep", name="rrep", bufs=4)
            nc.scalar.copy(out, rp)
            return out

        # ---------------------------------------------------------- layers
        for l in range(NL):
            w_in_t = [wpool.tile([P, 2 * DH], BF16, tag=f"w_in{k}",
                                 name=f"w_in{k}") for k in range(FT)]
            for k in range(FT):
                nc.sync.dma_start(w_in_t[k], w_in[l, k * P:(k + 1) * P, :])
            wsum_t = wpool.tile([1, 2 * DH], BF16, tag="wsum", name="wsum")
            nc.sync.dma_start(wsum_t, wsum_neg[l])
            b_in_t = [wpool.tile([P, 1], F32, tag=f"b_in{e}", name=f"b_in{e}")
                      for e in range(2 * DH // P)]
            for e in range(2 * DH // P):
                nc.sync.dma_start(b_in_t[e], b_in[l, e * P:(e + 1) * P, :])
            dcw_t = [wpool.tile([P, DCONV], F32, tag=f"dcw{g}", name=f"dcw{g}")
                     for g in range(G)]
            dcb_t = [wpool.tile([P, 1], F32, tag=f"dcb{g}", name=f"dcb{g}")
                     for g in range(G)]
            w_xp_t = [wpool.tile([P, XP], BF16, tag=f"w_xp{g}", name=f"w_xp{g}")
                      for g in range(G)]
            b_dt_t = [wpool.tile([P, 1], F32, tag=f"b_dt{g}", name=f"b_dt{g}")
                      for g in range(G)]
            ac_t = [wpool.tile([P, DS], F32, tag=f"ac{g}", name=f"ac{g}")
                    for g in range(G)]
            dpar_t = [wpool.tile([P, 1], F32, tag=f"dpar{g}", name=f"dpar{g}")
                      for g in range(G)]
            w_out_t = [wpool.tile([P, D], BF16, tag=f"w_out{g}", name=f"w_out{g}")
                       for g in range(G)]
            for g in range(G):
                s = slice(g * P, (g + 1) * P)
                nc.sync.dma_start(dcw_t[g], dconv_wt[l, s, :])
                nc.sync.dma_start(dcb_t[g], dconv_bt[l, s, :])
                nc.sync.dma_start(w_xp_t[g], w_xp[l, s, :])
                nc.sync.dma_start(b_dt_t[g], b_dt[l, s, :])
                nc.sync.dma_start(ac_t[g], a_cols[l, s, :])
                nc.sync.dma_start(dpar_t[g], d_par[l, s, :])
                nc.sync.dma_start(w_out_t[g], w_out[l, s, :])
            w_dt_t = wpool.tile([R, DH], BF16, tag="w_dt", name="w_dt")
            nc.sync.dma_start(w_dt_t, w_dt[l])

            # full-T concat buffers (xs written in-place over xraw after dconv)
            # pad to 4 so the xs view starts at an even element offset (DVE 2x)
            PAD = 4
            xraw_c = [big.tile([P, PAD + T], BF16, tag=f"xrc{g}",
                               name=f"xrc{g}") for g in range(G)]
            zs_c = [big.tile([P, T], BF16, tag=f"zsc{g}", name=f"zsc{g}")
                    for g in range(G)]
            for g in range(G):
                nc.vector.memset(xraw_c[g][:, 0:PAD], 0.0)

            # ===== phase A: stats for all chunks first (cross-chunk pipelining)
            srows = []
            for ci in range(NC):
                t0 = ci * TC
                hc, s1, s2 = ln_stats(slice(t0, t0 + TC), ci)
                rstd_bf = small.tile([1, TC], BF16, tag="rb", name="rb", bufs=1)
                nc.scalar.copy(rstd_bf, s2)
                mr_row = small.tile([1, TC], BF16, tag=f"mr{ci}", name="mr",
                                    bufs=1)
                nc.vector.tensor_tensor(mr_row, s1, s2, op=OP.mult)
                srows.append((hc, mr_row, rep_row(rstd_bf)))
            # ===== in_proj per chunk into concat buffers
            for ci in range(NC):
                t0 = ci * TC
                sl = slice(t0, t0 + TC)
                hc, mr_row, rrep = srows[ci]
                hcn = hc
                for f in range(FT):
                    nc.vector.tensor_tensor(hcn[f], hc[f], rrep, op=OP.mult)
                for half in range(2):
                    for g in range(G):
                        e0 = half * DH + g * P
                        pt = pa.tile([P, TC], F32, tag="mm", name="mm")
                        for k in range(FT):
                            nc.tensor.matmul(pt, w_in_t[k][:, e0:e0 + P],
                                             hcn[k], start=(k == 0), stop=False)
                        nc.tensor.matmul(pt, wsum_t[:, e0:e0 + P], mr_row,
                                         start=False, stop=True)
                        bia = b_in_t[e0 // P]
                        if half == 0:
                            nc.scalar.activation(
                                xraw_c[g][:, PAD + t0:PAD + t0 + TC],
                                pt, AF.Identity, bias=bia, scale=1.0)
                        else:
                            nc.scalar.activation(zs_c[g][:, sl], pt, AF.Silu,
                                                 bias=bia, scale=1.0)

            # ============ dconv on gpsimd, per T-half (in-place silu -> xs)
            TH = T // 2
            xs_c = []
            for g in range(G):
                for hh in range(2):
                    o0 = hh * TH
                    tmp = small.tile([P, TH], BF16, tag="dctmp", name="dctmp",
                                     bufs=2)
                    nc.vector.tensor_scalar(tmp, xraw_c[g][:, 1 + o0:1 + o0 + TH],
                                            dcw_t[g][:, 0:1], None, OP.mult)
                    for j in range(1, DCONV):
                        tmp2 = small.tile([P, TH], BF16, tag="dctmp",
                                          name="dctmp", bufs=2)
                        nc.vector.scalar_tensor_tensor(
                            tmp2, xraw_c[g][:, 1 + j + o0:1 + j + o0 + TH],
                            dcw_t[g][:, j:j + 1], tmp, op0=OP.mult, op1=OP.add)
                        tmp = tmp2
                    nc.scalar.activation(xraw_c[g][:, PAD + o0:PAD + o0 + TH],
                                         tmp, AF.Silu, bias=dcb_t[g],
                                         scale=1.0)
                xs_c.append(xraw_c[g][:, PAD:PAD + T])

            # ===== x_proj partials + pair AllReduce per T-half (bf16)
            T2 = T // 2
            for hf in range(2):
                for cj in range(2):
                    t0 = (2 * hf + cj) * TC
                    pt = pa.tile([P, TC], F32, tag="mm", name="mm")
                    for g in range(G):
                        nc.tensor.matmul(pt[0:XP, :], w_xp_t[g],
                                         xs_c[g][:, t0:t0 + TC],
                                         start=(g == 0), stop=(g == G - 1))
                    dbc_p = small.tile([XP, TC], BF16, tag="dbc_p",
                                       name="dbc_p", bufs=1)
                    nc.scalar.copy(dbc_p, pt[0:XP, :])
                    nc.sync.dma_start(
                        cc_dbc_i[2 * l + hf][:, cj * TC:(cj + 1) * TC], dbc_p)
                nc.gpsimd.collective_compute(
                    "AllReduce", OP.add, replica_groups=GROUPS,
                    ins=[cc_dbc_i[2 * l + hf][:, :]],
                    outs=[cc_dbc_o[2 * l + hf][:, :]])

            dt_c = [big.tile([P, T], BF16, tag=f"dtc{g}", name=f"dtc{g}")
                    for g in range(G)]
            dtu_c = [big.tile([P, T], BF16, tag=f"dtuc{g}", name=f"dtuc{g}")
                     for g in range(G)]
            fin = [big.tile([P, DS], F32, tag=f"fin{g}", name=f"fin{g}")
                   for g in range(G)]

            def emit_dt(hf):
                """dt = softplus(w_dt @ dbc_dt + b); dtu = dt*xs (one T-half)."""
                h0 = hf * T2
                hsl = slice(h0, h0 + T2)
                cc = cc_dbc_o[2 * l + hf]
                dbc_dt = small.tile([R, T2], BF16, tag="dbc_dt",
                                    name="dbc_dt", bufs=1)
                nc.sync.dma_start(dbc_dt, cc[0:R, :])
                # softplus: all exps first, then all lns in-place (1 table swap)
                for g in range(G):
                    for cj in range(2):
                        ptd = pb.tile([P, TC], F32, tag="mm", name="mm")
                        nc.tensor.matmul(ptd, w_dt_t[:, g * P:(g + 1) * P],
                                         dbc_dt[:, cj * TC:(cj + 1) * TC],
                                         start=True, stop=True)
                        nc.scalar.activation(
                            dt_c[g][:, h0 + cj * TC:h0 + (cj + 1) * TC], ptd,
                            AF.Exp, bias=b_dt_t[g], scale=1.0)
                for g in range(G):
                    for cj in range(2):
                        sl2 = slice(h0 + cj * TC, h0 + (cj + 1) * TC)
                        nc.scalar.activation(dt_c[g][:, sl2], dt_c[g][:, sl2],
                                             AF.Ln, bias=1.0, scale=1.0)
                for g in range(G):
                    nc.vector.tensor_tensor(dtu_c[g][:, hsl], dt_c[g][:, hsl],
                                            xs_c[g][:, hsl], op=OP.mult)

            emit_dt(0)
            for hf in range(2):
                h0 = hf * T2
                hsl = slice(h0, h0 + T2)
                cc = cc_dbc_o[2 * l + hf]
                # ---- scan blocks: g-pairs share B/C fetches
                for gp in ((0, 1), (2, 3)):
                    # overlap: emit next half's dt work mid-way so Act/PE
                    # compute it during this half's scans
                    if hf == 0 and gp == (2, 3):
                        emit_dt(1)
                    ypgs = {}
                    for g in gp:
                        yp = pyac.tile([P, T2], F32, tag=f"y{g % 2}",
                                       name=f"y{g % 2}")
                        for cj in range(2):
                            xsD = small.tile([P, TC], BF16, tag="xsD",
                                             name="xsD", bufs=1)
                            nc.vector.tensor_scalar(
                                xsD, xs_c[g][:, h0 + cj * TC:h0 + (cj + 1) * TC],
                                dpar_t[g], None, OP.mult)
                            nc.tensor.matmul(yp[:, cj * TC:(cj + 1) * TC],
                                             identb, xsD, start=True,
                                             stop=False)
                        ypgs[g] = yp

                    def fetch_q(q):
                        """One DMA per 2-row pair of B rows / C rows."""
                        bt = bc.tile([P, 2 * T2], BF16, tag=f"bq{q % 2}",
                                     name=f"bq{q % 2}")
                        ct = bc.tile([P, 2 * T2], BF16, tag=f"cq{q % 2}",
                                     name=f"cq{q % 2}")
                        bdst = bass.AP(tensor=bt.tensor, offset=bt.offset,
                                       ap=[list(bt.ap[0]), [T2, 2], [1, T2]])
                        cdst = bass.AP(tensor=ct.tensor, offset=ct.offset,
                                       ap=[list(ct.ap[0]), [T2, 2], [1, T2]])
                        nc.scalar.dma_start(
                            bdst, bcast_rows(cc[R + 2 * q:R + 2 * q + 2, :]),
                            single_packet=True)
                        nc.gpsimd.dma_start(
                            cdst,
                            bcast_rows(cc[R + DS + 2 * q:R + DS + 2 * q + 2, :]),
                            single_packet=True)
                        return bt, ct

                    bcache = {0: fetch_q(0), 1: fetch_q(1)}
                    for q in range(8):
                        bt_q, ct_q = bcache[q % 2]
                        for ni in range(2):
                            if ni == 1 and 0 < q < 7:
                                # prefetch q+1 here so the DMA transfer
                                # overlaps ni=1's scans, not the cm/b_t ops
                                bcache[(q + 1) % 2] = fetch_q(q + 1)
                            n = 2 * q + ni
                            brep = bt_q[:, ni * T2:(ni + 1) * T2]
                            crep = ct_q[:, ni * T2:(ni + 1) * T2]
                            hscs = {}
                            for g in gp:
                                a_t = scanp.tile([P, T2], BF16, tag="sa",
                                                 name="sa", bufs=2)
                                nc.scalar.activation(a_t, dt_c[g][:, hsl],
                                                     AF.Exp,
                                                     scale=ac_t[g][:, n:n + 1])
                                b_t = scanp.tile([P, T2], BF16, tag="sb",
                                                 name="sb")
                                nc.vector.tensor_tensor(b_t, dtu_c[g][:, hsl],
                                                        brep, op=OP.mult)
                                hsc = scanp.tile([P, T2], BF16, tag="sh",
                                                 name="sh")
                                init = 0.0 if hf == 0 else fin[g][:, n:n + 1]
                                nc.vector.tensor_tensor_scan(hsc, a_t, b_t,
                                                             init,
                                                             op0=OP.mult,
                                                             op1=OP.add)
                                if hf == 0:
                                    nc.vector.tensor_copy(fin[g][:, n:n + 1],
                                                          hsc[:, T2 - 1:T2])
                                hscs[g] = hsc
                            # cm after both scans (avoid read-after-write
                            # stall on hsc); one per engine per n
                            for gi, g in enumerate(gp):
                                cm = scanp.tile([P, T2], BF16, tag="sm",
                                                name="sm")
                                eng = nc.gpsimd if ((n + gi) % 2 == 1) \
                                    else nc.vector
                                eng.tensor_tensor(cm, hscs[g], crep,
                                                  op=OP.mult)
                                for cj in range(2):
                                    nc.tensor.matmul(
                                        ypgs[g][:, cj * TC:(cj + 1) * TC],
                                        identb, cm[:, cj * TC:(cj + 1) * TC],
                                        start=False, stop=(n == DS - 1))
                    # gating into dtu_c (dead after n loop)
                    for g in gp:
                        nc.vector.tensor_tensor(dtu_c[g][:, hsl], ypgs[g],
                                                zs_c[g][:, hsl], op=OP.mult)

                # ---- out_proj + AR_y + residual for this half's chunks
                for cj in range(2):
                    ci = 2 * hf + cj
                    t0 = ci * TC
                    sl = slice(t0, t0 + TC)
                    ccidx = l * NC + ci
                    for f in range(FT):
                        po = pb.tile([P, TC], F32, tag="mm", name="mm")
                        for g in range(G):
                            nc.tensor.matmul(po,
                                             w_out_t[g][:, f * P:(f + 1) * P],
                                             dtu_c[g][:, sl], start=(g == 0),
                                             stop=(g == G - 1))
                        ot = small.tile([P, TC], BF16, tag="oout", name="oout")
                        nc.scalar.copy(ot, po)
                        nc.sync.dma_start(cc_y_i[ccidx][f * P:(f + 1) * P, :],
                                          ot)
                    nc.gpsimd.collective_compute(
                        "AllReduce", OP.add, replica_groups=GROUPS,
                        ins=[cc_y_i[ccidx][:, :]], outs=[cc_y_o[ccidx][:, :]])
                    for f in range(FT):
                        yfull = scanp.tile([P, TC], BF16, tag="yfull",
                                           name="yfull", bufs=2)
                        nc.sync.dma_start(yfull,
                                          cc_y_o[ccidx][f * P:(f + 1) * P, :])
                        nc.gpsimd.tensor_tensor(h[f][:, sl], h[f][:, sl],
                                                yfull, op=OP.add)

        # ------------------------------------------- final LN + transpose out
        fn_t = [persist.tile([P, 2], F32, name=f"fn{f}") for f in range(FT)]
        for f in range(FT):
            nc.sync.dma_start(fn_t[f], fn_wb[f * P:(f + 1) * P, :])
        for ci in range(NC):
            t0 = ci * TC
            sl = slice(t0, t0 + TC)
            hc, s1, s2 = ln_stats(sl, ci)
            s1_bf = small.tile([1, TC], BF16, tag=f"mr{ci}", name="mr", bufs=1)
            nc.vector.tensor_copy(s1_bf, s1)
            rstd_bf = small.tile([1, TC], BF16, tag=f"rb{ci}", name="rb",
                                 bufs=1)
            nc.vector.tensor_copy(rstd_bf, s2)
            mrep = rep_row(s1_bf)
            rrep = rep_row(rstd_bf)
            hn = []
            for f in range(FT):
                t2 = big.tile([P, TC], F32, tag=f"fhn{f}", name=f"hn{f}")
                nc.vector.tensor_tensor(t2, hc[f], mrep, op=OP.subtract)
                nc.vector.tensor_tensor(t2, t2, rrep, op=OP.mult)
                nc.vector.tensor_scalar(t2, t2, fn_t[f][:, 0:1],
                                        fn_t[f][:, 1:2], OP.mult, OP.add)
                hn.append(t2)
            for tb in range(TC // P):
                ht = small.tile([P, D], BF16, tag="ht", name="ht", bufs=1)
                for f in range(FT):
                    pt = pa.tile([P, TC], F32, tag="mm", name="mm")
                    nc.tensor.transpose(pt[:, 0:P], hn[f][:, tb * P:(tb + 1) * P],
                                        identf)
                    nc.scalar.copy(ht[:, f * P:(f + 1) * P], pt[:, 0:P])
                tglob = t0 + tb * P
                src = ht[:, :]
                rep_in = bass.AP(tensor=src.tensor, offset=src.offset,
                                 ap=[list(src.ap[0]), [0, STRIDE],
                                     list(src.ap[1])])
                dst = y_out[STRIDE * tglob:STRIDE * (tglob + P), :]
                dst3 = dst.rearrange("(t r) d -> t r d", r=STRIDE)
                nc.sync.dma_start(dst3, rep_in)

    nc.compile()
    return nc


# ================================================================ host side
def make_core_inputs(inputs, T=2048, NL=4):
    x = np.asarray(inputs["x"], np.float32)
    conv_w = np.asarray(inputs["conv_w"], np.float32)
    conv_b = np.asarray(inputs["conv_b"], np.float32)
    in_proj_w = np.asarray(inputs["in_proj_w"], np.float32)
    dconv_w = np.asarray(inputs["dconv_w"], np.float32)
    dconv_b = np.asarray(inputs["dconv_b"], np.float32)
    x_proj_w = np.asarray(inputs["x_proj_w"], np.float32)
    dt_proj_w = np.asarray(inputs["dt_proj_w"], np.float32)
    dt_proj_b = np.asarray(inputs["dt_proj_b"], np.float32)
    A_log = np.asarray(inputs["A_log"], np.float32)
    D_param = np.asarray(inputs["D_param"], np.float32)
    out_proj_w = np.asarray(inputs["out_proj_w"], np.float32)
    ln_w = np.asarray(inputs["ln_w"], np.float32)
    ln_b = np.asarray(inputs["ln_b"], np.float32)
    fn_w = np.asarray(inputs["fn_w"], np.float32)
    fn_b = np.asarray(inputs["fn_b"], np.float32)

    Bn = x.shape[0]
    di = x.shape[2]
    dmodel = conv_w.shape[0]
    dinner = in_proj_w.shape[1] // 2
    dh = dinner // 2

    xpad = np.concatenate([np.zeros((Bn, KF - 1, di), np.float32), x], axis=1)
    idx = np.arange(T)[:, None] * STRIDE + np.arange(KF)[None, :]
    xcat = xpad[:, idx, :].reshape(Bn, T, KF * di)
    xcatT = np.ascontiguousarray(xcat.transpose(0, 2, 1))
    wconv = np.ascontiguousarray(conv_w.transpose(2, 1, 0).reshape(KF * di, dmodel))

    A = -np.exp(A_log)

    per_core = []
    for c in range(8):
        b, j = c // 2, c % 2
        sl = slice(j * dh, (j + 1) * dh)
        w_in_l, b_in_l, w_out_l, w_xp_l, wsum_l = [], [], [], [], []
        for l in range(NL):
            Wx = in_proj_w[l, :dinner][sl] * ln_w[l][None, :]
            Wz = in_proj_w[l, dinner:][sl] * ln_w[l][None, :]
            wl = np.concatenate([Wx.T, Wz.T], axis=1)
            w_in_l.append(wl)
            wsum_l.append(-wl.sum(axis=0, keepdims=True))
            bx = in_proj_w[l, :dinner][sl] @ ln_b[l]
            bz = in_proj_w[l, dinner:][sl] @ ln_b[l]
            b_in_l.append(np.concatenate([bx, bz])[:, None])
            w_out_l.append(out_proj_w[l][:, sl].T)
            w_xp_l.append(np.ascontiguousarray(x_proj_w[l][:, sl].T))
        d = dict(
            xcatT=xcatT[b],
            wconv=wconv,
            conv_bias=conv_b[:, None],
            w_in=np.stack(w_in_l),
            b_in=np.stack(b_in_l),
            wsum_neg=np.stack(wsum_l),
            dconv_wt=dconv_w[:, sl, :],
            dconv_bt=dconv_b[:, sl, None],
            w_xp=np.stack(w_xp_l),
            w_dt=np.ascontiguousarray(dt_proj_w[:, sl, :].transpose(0, 2, 1)),
            b_dt=dt_proj_b[:, sl, None],
            a_cols=A[:, sl, :],
            d_par=D_param[:, sl, None],
            w_out=np.stack(w_out_l),
            fn_wb=np.stack([fn_w, fn_b], axis=1),
            identb_bf=np.eye(P, dtype=np.float32),
        )
        per_core.append(d)
    return per_core


def cast_core_inputs(nc, per_core):
    import concourse.mybir as mybir
    want = {}
    for alloc in nc.m.functions[0].allocations:
        if getattr(alloc, "kind", None) == "ExternalInput":
            want[alloc.memorylocations[0].name] = mybir.dt.np(alloc.dtype)
    return [{k: np.ascontiguousarray(np.asarray(v).astype(want[k]))
             for k, v in d.items() if k in want} for d in per_core]


_PROGRAM_CACHE = {}


def get_program(T=2048, NL=4, TC=512):
    key = (T, NL, TC)
    if key not in _PROGRAM_CACHE:
        _PROGRAM_CACHE[key] = build_program(T, NL, TC)
    return _PROGRAM_CACHE[key]


def kernel(**inputs):
    from concourse.bass_utils import run_bass_kernel_spmd
    T = inputs["x"].shape[1] // STRIDE
    NL = inputs["in_proj_w"].shape[0]
    nc = get_program(T, NL)
    per_core = cast_core_inputs(nc, make_core_inputs(inputs, T, NL))
    res = run_bass_kernel_spmd(nc, per_core, core_ids=list(range(8)))
    Bn = inputs["x"].shape[0]
    y = np.stack([res.results[2 * b]["y_out"] for b in range(Bn)])
    return y.astype(np.float32)



# revision 28
# speedup vs baseline: 1.1010x; 1.0978x over previous
"""Trainium2 Bass kernel for nn_ConvBranch: strided-conv front end + 4 Mamba
layers + final LN + x4 upsample.

Sharding (8 cores): core c = (batch b = c//2, d_inner half j = c%2).
Each core: its batch, full sequence T=2048 (post-conv), full d_model=512,
its 512-channel half of d_inner=1024.  Contractions over d_inner (x_proj,
out_proj) produce partial sums -> pair AllReduce ([0,1],[2,3],[4,5],[6,7])
in bf16.

v2 layout/perf notes:
- B/C rows for the scan are broadcast to 128 partitions by stride-0 DMA
  reads from the AllReduce output in DRAM (no PE one-hot matmuls, no
  PSUM->SBUF copies).
- y = sum_n h_n*C_n accumulated on the PE via bf16 identity matmuls into
  PSUM (removes ~1000 DVE/GPSIMD adds).
- LN: stats from a bf16 copy of h; mean folded into in_proj as a rank-1
  correction (host-precomputed -colsum(W)), rstd broadcast via one K=1
  matmul.  All GEMMs bf16 (1 cyc/row).
- Per layer: phase A (LN+in_proj+dconv+x_proj+AllReduce for all chunks)
  then phase B (dt+scan+gate+out_proj+per-chunk y AllReduce) so collective
  latency overlaps compute.
"""

import sys

import numpy as np

sys.path.insert(0, "/opt/trn_rl_repo")

B_ = 4
D_IN = 256
D = 512          # d_model
STRIDE = 4
KF = 8           # front conv kernel
DS = 16          # d_state
DCONV = 4
DI = 1024        # d_inner
DH = DI // 2     # per-core d_inner half
R = 32           # dt_rank
LN_EPS = 1e-5
P = 128
G = DH // P      # 4
FT = D // P      # 4
XP = 64          # x_proj rows: [dt 0:32 | B 32:48 | C 48:64]
GROUPS = [[0, 1], [2, 3], [4, 5], [6, 7]]


# ====================================================================== build
def build_program(T=2048, NL=4, TC=512):
    import contextlib

    import concourse.bacc as bacc
    import concourse.bass as bass
    import concourse.mybir as mybir
    from concourse.tile import TileContext

    F32 = mybir.dt.float32
    BF16 = mybir.dt.bfloat16
    AF = mybir.ActivationFunctionType
    OP = mybir.AluOpType

    TC = min(TC, T)
    NC = T // TC
    assert TC <= 512
    T_IN = T * STRIDE

    nc = bacc.Bacc("TRN2", target_bir_lowering=False, debug=False,
                   enable_asserts=False, num_devices=8)

    xcatT = nc.dram_tensor("xcatT", [2 * STRIDE * D_IN, T], BF16, kind="ExternalInput")
    wconv = nc.dram_tensor("wconv", [2 * STRIDE * D_IN, D], BF16, kind="ExternalInput")
    conv_bias = nc.dram_tensor("conv_bias", [D, 1], F32, kind="ExternalInput")
    w_in = nc.dram_tensor("w_in", [NL, D, 2 * DH], BF16, kind="ExternalInput")
    b_in = nc.dram_tensor("b_in", [NL, 2 * DH, 1], F32, kind="ExternalInput")
    wsum_neg = nc.dram_tensor("wsum_neg", [NL, 1, 2 * DH], BF16, kind="ExternalInput")
    dconv_wt = nc.dram_tensor("dconv_wt", [NL, DH, DCONV], F32, kind="ExternalInput")
    dconv_bt = nc.dram_tensor("dconv_bt", [NL, DH, 1], F32, kind="ExternalInput")
    w_xp = nc.dram_tensor("w_xp", [NL, DH, XP], BF16, kind="ExternalInput")
    w_dt = nc.dram_tensor("w_dt", [NL, R, DH], BF16, kind="ExternalInput")
    b_dt = nc.dram_tensor("b_dt", [NL, DH, 1], F32, kind="ExternalInput")
    a_cols = nc.dram_tensor("a_cols", [NL, DH, DS], F32, kind="ExternalInput")
    d_par = nc.dram_tensor("d_par", [NL, DH, 1], F32, kind="ExternalInput")
    w_out = nc.dram_tensor("w_out", [NL, DH, D], BF16, kind="ExternalInput")
    fn_wb = nc.dram_tensor("fn_wb", [D, 2], F32, kind="ExternalInput")
    identb_bf = nc.dram_tensor("identb_bf", [P, P], BF16, kind="ExternalInput")
    y_out = nc.dram_tensor("y_out", [T_IN, D], BF16, kind="ExternalOutput")

    NCK = NL * NC
    cc_dbc_i = [nc.dram_tensor(f"cc_dbc_i{k}", [XP, T // 2], BF16) for k in range(2 * NL)]
    cc_dbc_o = [nc.dram_tensor(f"cc_dbc_o{k}", [XP, T // 2], BF16) for k in range(2 * NL)]
    cc_y_i = [nc.dram_tensor(f"cc_y_i{k}", [D, TC], BF16) for k in range(NCK)]
    cc_y_o = [nc.dram_tensor(f"cc_y_o{k}", [D, TC], BF16) for k in range(NCK)]

    def bcast_rows(dram_rows):
        """[R, W] DRAM rows -> stride-0 AP readable as [P, R, W]."""
        return bass.AP(tensor=dram_rows.tensor, offset=dram_rows.offset,
                       ap=[[0, P]] + [list(d) for d in dram_rows.ap])

    with TileContext(nc) as tc, contextlib.ExitStack() as ctx:
        persist = ctx.enter_context(tc.tile_pool(name="persist", bufs=1))
        wpool = ctx.enter_context(tc.tile_pool(name="wpool", bufs=1))
        big = ctx.enter_context(tc.tile_pool(name="big", bufs=1))
        scanp = ctx.enter_context(tc.tile_pool(name="scanp", bufs=2))
        bc = ctx.enter_context(tc.tile_pool(name="bc", bufs=1))
        small = ctx.enter_context(tc.tile_pool(name="small", bufs=2))

        ones_col_bf = persist.tile([P, 1], BF16)
        nc.vector.memset(ones_col_bf, 1.0 / D)
        ones_row_bf = persist.tile([1, P], BF16)
        nc.vector.memset(ones_row_bf, 1.0)
        identb = persist.tile([P, P], BF16)
        nc.sync.dma_start(identb, identb_bf[:, :])
        identf = persist.tile([P, P], F32)
        nc.scalar.copy(identf, identb)
        eps_t = persist.tile([P, 1], F32)
        nc.vector.memset(eps_t, LN_EPS)

        h = [persist.tile([P, T], F32, name=f"h{f}") for f in range(FT)]

        # ------------------------------------------------- front conv + GELU
        with tc.tile_pool(name="convp", bufs=1) as convp, \
             tc.tile_pool(name="convx", bufs=4) as convx, \
             tc.tile_pool(name="convps", bufs=4, space="PSUM") as convps:
            K16 = (2 * STRIDE * D_IN) // P
            cb = []
            for f in range(FT):
                cbf = convp.tile([P, 1], F32, name=f"cb{f}")
                nc.sync.dma_start(cbf, conv_bias[f * P:(f + 1) * P, :])
                cb.append(cbf)
            for c in range(T // TC):
                pts = [convps.tile([P, TC], F32, tag="mm", name="mm")
                       for _ in range(FT)]
                for k in range(K16):
                    wt = convx.tile([P, D], BF16, tag="wc", name="wc", bufs=2)
                    nc.sync.dma_start(wt, wconv[k * P:(k + 1) * P, :])
                    xt = convx.tile([P, TC], BF16, tag="xcat", name="xcat", bufs=2)
                    nc.sync.dma_start(xt, xcatT[k * P:(k + 1) * P,
                                                c * TC:(c + 1) * TC])
                    for f in range(FT):
                        nc.tensor.matmul(pts[f], wt[:, f * P:(f + 1) * P],
                                         xt, start=(k == 0), stop=(k == K16 - 1))
                for f in range(FT):
                    nc.scalar.activation(h[f][:, c * TC:(c + 1) * TC], pts[f],
                                         AF.Gelu, bias=cb[f], scale=1.0)

        pa = ctx.enter_context(tc.tile_pool(name="pa", bufs=2, space="PSUM"))
        pb = ctx.enter_context(tc.tile_pool(name="pb", bufs=2, space="PSUM"))
        pyac = ctx.enter_context(tc.tile_pool(name="pyac", bufs=1, space="PSUM"))

        def ln_stats(sl, ci=0):
            """bf16 copy of h chunk + mean/rstd rows; returns (hc, s1, rstd)."""
            stat = pa.tile([P, TC], F32, tag="mm", name="stat")
            hc = []
            for f in range(FT):
                c = big.tile([P, TC], BF16, tag=f"hc{f}_{ci}", name=f"hc{f}")
                nc.scalar.copy(c, h[f][:, sl])
                hc.append(c)
                nc.tensor.matmul(stat[0:1, :], ones_col_bf, c,
                                 start=(f == 0), stop=(f == FT - 1))
            for f in range(FT):
                q = big.tile([P, TC], BF16, tag="hsq", name="hsq", bufs=1)
                nc.scalar.activation(q, hc[f], AF.Square)
                nc.tensor.matmul(stat[32:33, :], ones_col_bf, q,
                                 start=(f == 0), stop=(f == FT - 1))
            s1 = small.tile([1, TC], F32, tag="s1", name="s1", bufs=1)
            nc.scalar.copy(s1, stat[0:1, :])               # mean (ones = 1/D)
            msq = small.tile([1, TC], F32, tag="msq", name="msq", bufs=1)
            nc.scalar.activation(msq, s1, AF.Square)
            s2 = small.tile([1, TC], F32, tag="s2", name="s2", bufs=1)
            nc.vector.tensor_tensor(s2, stat[32:33, :], msq, op=OP.subtract)
            nc.scalar.activation(s2, s2, AF.Ln, bias=eps_t[0:1, :], scale=1.0)
            nc.scalar.activation(s2, s2, AF.Exp, scale=-0.5)  # rstd
            return hc, s1, s2

        def rep_row(row_bf):
            """Broadcast a [1, TC] bf16 row to a [P, TC] bf16 tile via PE."""
            rp = pa.tile([P, TC], F32, tag="mm", name="rep")
            nc.tensor.matmul(rp, ones_row_bf, row_bf, start=True, stop=True)
            out = big.tile([P, TC], BF16, tag="rrep", name="rrep", bufs=4)
            nc.scalar.copy(out, rp)
            return out

        # ---------------------------------------------------------- layers
        for l in range(NL):
            w_in_t = [wpool.tile([P, 2 * DH], BF16, tag=f"w_in{k}",
                                 name=f"w_in{k}") for k in range(FT)]
            for k in range(FT):
                nc.sync.dma_start(w_in_t[k], w_in[l, k * P:(k + 1) * P, :])
            wsum_t = wpool.tile([1, 2 * DH], BF16, tag="wsum", name="wsum")
            nc.sync.dma_start(wsum_t, wsum_neg[l])
            b_in_t = [wpool.tile([P, 1], F32, tag=f"b_in{e}", name=f"b_in{e}")
                      for e in range(2 * DH // P)]
            for e in range(2 * DH // P):
                nc.sync.dma_start(b_in_t[e], b_in[l, e * P:(e + 1) * P, :])
            dcw_t = [wpool.tile([P, DCONV], F32, tag=f"dcw{g}", name=f"dcw{g}")
                     for g in range(G)]
            dcb_t = [wpool.tile([P, 1], F32, tag=f"dcb{g}", name=f"dcb{g}")
                     for g in range(G)]
            w_xp_t = [wpool.tile([P, XP], BF16, tag=f"w_xp{g}", name=f"w_xp{g}")
                      for g in range(G)]
            b_dt_t = [wpool.tile([P, 1], F32, tag=f"b_dt{g}", name=f"b_dt{g}")
                      for g in range(G)]
            ac_t = [wpool.tile([P, DS], F32, tag=f"ac{g}", name=f"ac{g}")
                    for g in range(G)]
            dpar_t = [wpool.tile([P, 1], F32, tag=f"dpar{g}", name=f"dpar{g}")
                      for g in range(G)]
            w_out_t = [wpool.tile([P, D], BF16, tag=f"w_out{g}", name=f"w_out{g}")
                       for g in range(G)]
            for g in range(G):
                s = slice(g * P, (g + 1) * P)
                nc.sync.dma_start(dcw_t[g], dconv_wt[l, s, :])
                nc.sync.dma_start(dcb_t[g], dconv_bt[l, s, :])
                nc.sync.dma_start(w_xp_t[g], w_xp[l, s, :])
                nc.sync.dma_start(b_dt_t[g], b_dt[l, s, :])
                nc.sync.dma_start(ac_t[g], a_cols[l, s, :])
                nc.sync.dma_start(dpar_t[g], d_par[l, s, :])
                nc.sync.dma_start(w_out_t[g], w_out[l, s, :])
            w_dt_t = wpool.tile([R, DH], BF16, tag="w_dt", name="w_dt")
            nc.sync.dma_start(w_dt_t, w_dt[l])

            # full-T concat buffers (xs written in-place over xraw after dconv)
            # pad to 4 so the xs view starts at an even element offset (DVE 2x)
            PAD = 4
            xraw_c = [big.tile([P, PAD + T], BF16, tag=f"xrc{g}",
                               name=f"xrc{g}") for g in range(G)]
            zs_c = [big.tile([P, T], BF16, tag=f"zsc{g}", name=f"zsc{g}")
                    for g in range(G)]
            for g in range(G):
                nc.vector.memset(xraw_c[g][:, 0:PAD], 0.0)

            # ===== phase A: stats for all chunks first (cross-chunk pipelining)
            srows = []
            for ci in range(NC):
                t0 = ci * TC
                hc, s1, s2 = ln_stats(slice(t0, t0 + TC), ci)
                rstd_bf = small.tile([1, TC], BF16, tag="rb", name="rb", bufs=1)
                nc.scalar.copy(rstd_bf, s2)
                mr_row = small.tile([1, TC], BF16, tag=f"mr{ci}", name="mr",
                                    bufs=1)
                nc.vector.tensor_tensor(mr_row, s1, s2, op=OP.mult)
                srows.append((hc, mr_row, rep_row(rstd_bf)))
            # ===== in_proj per chunk into concat buffers
            for ci in range(NC):
                t0 = ci * TC
                sl = slice(t0, t0 + TC)
                hc, mr_row, rrep = srows[ci]
                hcn = hc
                for f in range(FT):
                    nc.vector.tensor_tensor(hcn[f], hc[f], rrep, op=OP.mult)
                for half in range(2):
                    for g in range(G):
                        e0 = half * DH + g * P
                        pt = pa.tile([P, TC], F32, tag="mm", name="mm")
                        for k in range(FT):
                            nc.tensor.matmul(pt, w_in_t[k][:, e0:e0 + P],
                                             hcn[k], start=(k == 0), stop=False)
                        nc.tensor.matmul(pt, wsum_t[:, e0:e0 + P], mr_row,
                                         start=False, stop=True)
                        bia = b_in_t[e0 // P]
                        if half == 0:
                            nc.scalar.activation(
                                xraw_c[g][:, PAD + t0:PAD + t0 + TC],
                                pt, AF.Identity, bias=bia, scale=1.0)
                        else:
                            nc.scalar.activation(zs_c[g][:, sl], pt, AF.Silu,
                                                 bias=bia, scale=1.0)

            # ============ dconv on gpsimd, per T-half (in-place silu -> xs)
            TH = T // 2
            xs_c = []
            for g in range(G):
                for hh in range(2):
                    o0 = hh * TH
                    tmp = small.tile([P, TH], BF16, tag="dctmp", name="dctmp",
                                     bufs=2)
                    nc.vector.tensor_scalar(tmp, xraw_c[g][:, 1 + o0:1 + o0 + TH],
                                            dcw_t[g][:, 0:1], None, OP.mult)
                    for j in range(1, DCONV):
                        tmp2 = small.tile([P, TH], BF16, tag="dctmp",
                                          name="dctmp", bufs=2)
                        nc.vector.scalar_tensor_tensor(
                            tmp2, xraw_c[g][:, 1 + j + o0:1 + j + o0 + TH],
                            dcw_t[g][:, j:j + 1], tmp, op0=OP.mult, op1=OP.add)
                        tmp = tmp2
                    nc.scalar.activation(xraw_c[g][:, PAD + o0:PAD + o0 + TH],
                                         tmp, AF.Silu, bias=dcb_t[g],
                                         scale=1.0)
                xs_c.append(xraw_c[g][:, PAD:PAD + T])

            # ===== x_proj partials + pair AllReduce per T-half (bf16)
            T2 = T // 2
            for hf in range(2):
                for cj in range(2):
                    t0 = (2 * hf + cj) * TC
                    pt = pa.tile([P, TC], F32, tag="mm", name="mm")
                    for g in range(G):
                        nc.tensor.matmul(pt[0:XP, :], w_xp_t[g],
                                         xs_c[g][:, t0:t0 + TC],
                                         start=(g == 0), stop=(g == G - 1))
                    dbc_p = small.tile([XP, TC], BF16, tag="dbc_p",
                                       name="dbc_p", bufs=1)
                    nc.scalar.copy(dbc_p, pt[0:XP, :])
                    nc.sync.dma_start(
                        cc_dbc_i[2 * l + hf][:, cj * TC:(cj + 1) * TC], dbc_p)
                nc.gpsimd.collective_compute(
                    "AllReduce", OP.add, replica_groups=GROUPS,
                    ins=[cc_dbc_i[2 * l + hf][:, :]],
                    outs=[cc_dbc_o[2 * l + hf][:, :]])

            dt_c = [big.tile([P, T], BF16, tag=f"dtc{g}", name=f"dtc{g}")
                    for g in range(G)]
            dtu_c = [big.tile([P, T], BF16, tag=f"dtuc{g}", name=f"dtuc{g}")
                     for g in range(G)]
            fin = [big.tile([P, DS], F32, tag=f"fin{g}", name=f"fin{g}")
                   for g in range(G)]

            def emit_dt(hf):
                """dt = softplus(w_dt @ dbc_dt + b); dtu = dt*xs (one T-half)."""
                h0 = hf * T2
                hsl = slice(h0, h0 + T2)
                cc = cc_dbc_o[2 * l + hf]
                dbc_dt = small.tile([R, T2], BF16, tag="dbc_dt",
                                    name="dbc_dt", bufs=1)
                nc.sync.dma_start(dbc_dt, cc[0:R, :])
                # softplus: all exps first, then all lns in-place (1 table swap)
                for g in range(G):
                    for cj in range(2):
                        ptd = pb.tile([P, TC], F32, tag="mm", name="mm")
                        nc.tensor.matmul(ptd, w_dt_t[:, g * P:(g + 1) * P],
                                         dbc_dt[:, cj * TC:(cj + 1) * TC],
                                         start=True, stop=True)
                        nc.scalar.activation(
                            dt_c[g][:, h0 + cj * TC:h0 + (cj + 1) * TC], ptd,
                            AF.Exp, bias=b_dt_t[g], scale=1.0)
                for g in range(G):
                    for cj in range(2):
                        sl2 = slice(h0 + cj * TC, h0 + (cj + 1) * TC)
                        nc.scalar.activation(dt_c[g][:, sl2], dt_c[g][:, sl2],
                                             AF.Ln, bias=1.0, scale=1.0)
                for g in range(G):
                    nc.vector.tensor_tensor(dtu_c[g][:, hsl], dt_c[g][:, hsl],
                                            xs_c[g][:, hsl], op=OP.mult)

            emit_dt(0)
            for hf in range(2):
                h0 = hf * T2
                hsl = slice(h0, h0 + T2)
                cc = cc_dbc_o[2 * l + hf]
                # ---- scan blocks: g-pairs share B/C fetches
                for gp in ((0, 1), (2, 3)):
                    # overlap: emit next half's dt work mid-way so Act/PE
                    # compute it during this half's scans
                    if hf == 0 and gp == (2, 3):
                        emit_dt(1)
                    ypgs = {}
                    for g in gp:
                        yp = pyac.tile([P, T2], F32, tag=f"y{g % 2}",
                                       name=f"y{g % 2}")
                        for cj in range(2):
                            xsD = small.tile([P, TC], BF16, tag="xsD",
                                             name="xsD", bufs=1)
                            nc.vector.tensor_scalar(
                                xsD, xs_c[g][:, h0 + cj * TC:h0 + (cj + 1) * TC],
                                dpar_t[g], None, OP.mult)
                            nc.tensor.matmul(yp[:, cj * TC:(cj + 1) * TC],
                                             identb, xsD, start=True,
                                             stop=False)
                        ypgs[g] = yp

                    def fetch_q(q):
                        """One DMA per 2-row pair of B rows / C rows."""
                        bt = bc.tile([P, 2 * T2], BF16, tag=f"bq{q % 2}",
                                     name=f"bq{q % 2}")
                        ct = bc.tile([P, 2 * T2], BF16, tag=f"cq{q % 2}",
                                     name=f"cq{q % 2}")
                        bdst = bass.AP(tensor=bt.tensor, offset=bt.offset,
                                       ap=[list(bt.ap[0]), [T2, 2], [1, T2]])
                        cdst = bass.AP(tensor=ct.tensor, offset=ct.offset,
                                       ap=[list(ct.ap[0]), [T2, 2], [1, T2]])
                        nc.scalar.dma_start(
                            bdst, bcast_rows(cc[R + 2 * q:R + 2 * q + 2, :]),
                            single_packet=True)
                        nc.gpsimd.dma_start(
                            cdst,
                            bcast_rows(cc[R + DS + 2 * q:R + DS + 2 * q + 2, :]),
                            single_packet=True)
                        return bt, ct

                    bcache = {0: fetch_q(0), 1: fetch_q(1)}
                    for q in range(8):
                        bt_q, ct_q = bcache[q % 2]
                        for ni in range(2):
                            if ni == 1 and 0 < q < 7:
                                # prefetch q+1 here so the DMA transfer
                                # overlaps ni=1's scans, not the cm/b_t ops
                                bcache[(q + 1) % 2] = fetch_q(q + 1)
                            n = 2 * q + ni
                            brep = bt_q[:, ni * T2:(ni + 1) * T2]
                            crep = ct_q[:, ni * T2:(ni + 1) * T2]
                            hscs = {}
                            for g in gp:
                                a_t = scanp.tile([P, T2], BF16, tag="sa",
                                                 name="sa", bufs=2)
                                nc.scalar.activation(a_t, dt_c[g][:, hsl],
                                                     AF.Exp,
                                                     scale=ac_t[g][:, n:n + 1])
                                b_t = scanp.tile([P, T2], BF16, tag="sb",
                                                 name="sb")
                                nc.vector.tensor_tensor(b_t, dtu_c[g][:, hsl],
                                                        brep, op=OP.mult)
                                hsc = scanp.tile([P, T2], BF16, tag="sh",
                                                 name="sh")
                                init = 0.0 if hf == 0 else fin[g][:, n:n + 1]
                                nc.vector.tensor_tensor_scan(hsc, a_t, b_t,
                                                             init,
                                                             op0=OP.mult,
                                                             op1=OP.add)
                                if hf == 0:
                                    nc.vector.tensor_copy(fin[g][:, n:n + 1],
                                                          hsc[:, T2 - 1:T2])
                                hscs[g] = hsc
                            # cm after both scans (avoid read-after-write
                            # stall on hsc); one per engine per n
                            for gi, g in enumerate(gp):
                                cm = scanp.tile([P, T2], BF16, tag="sm",
                                                name="sm")
                                nc.vector.tensor_tensor(cm, hscs[g], crep,
                                                        op=OP.mult)
                                for cj in range(2):
                                    nc.tensor.matmul(
                                        ypgs[g][:, cj * TC:(cj + 1) * TC],
                                        identb, cm[:, cj * TC:(cj + 1) * TC],
                                        start=False, stop=(n == DS - 1))
                    # gating into dtu_c (dead after n loop)
                    for g in gp:
                        nc.vector.tensor_tensor(dtu_c[g][:, hsl], ypgs[g],
                                                zs_c[g][:, hsl], op=OP.mult)

                # ---- out_proj + AR_y + residual for this half's chunks
                for cj in range(2):
                    ci = 2 * hf + cj
                    t0 = ci * TC
                    sl = slice(t0, t0 + TC)
                    ccidx = l * NC + ci
                    for f in range(FT):
                        po = pb.tile([P, TC], F32, tag="mm", name="mm")
                        for g in range(G):
                            nc.tensor.matmul(po,
                                             w_out_t[g][:, f * P:(f + 1) * P],
                                             dtu_c[g][:, sl], start=(g == 0),
                                             stop=(g == G - 1))
                        ot = small.tile([P, TC], BF16, tag="oout", name="oout")
                        nc.scalar.copy(ot, po)
                        nc.sync.dma_start(cc_y_i[ccidx][f * P:(f + 1) * P, :],
                                          ot)
                    nc.gpsimd.collective_compute(
                        "AllReduce", OP.add, replica_groups=GROUPS,
                        ins=[cc_y_i[ccidx][:, :]], outs=[cc_y_o[ccidx][:, :]])
                    for f in range(FT):
                        yfull = scanp.tile([P, TC], BF16, tag="yfull",
                                           name="yfull", bufs=2)
                        nc.sync.dma_start(yfull,
                                          cc_y_o[ccidx][f * P:(f + 1) * P, :])
                        nc.gpsimd.tensor_tensor(h[f][:, sl], h[f][:, sl],
                                                yfull, op=OP.add)

        # ------------------------------------------- final LN + transpose out
        fn_t = [persist.tile([P, 2], F32, name=f"fn{f}") for f in range(FT)]
        for f in range(FT):
            nc.sync.dma_start(fn_t[f], fn_wb[f * P:(f + 1) * P, :])
        for ci in range(NC):
            t0 = ci * TC
            sl = slice(t0, t0 + TC)
            hc, s1, s2 = ln_stats(sl, ci)
            s1_bf = small.tile([1, TC], BF16, tag=f"mr{ci}", name="mr", bufs=1)
            nc.vector.tensor_copy(s1_bf, s1)
            rstd_bf = small.tile([1, TC], BF16, tag=f"rb{ci}", name="rb",
                                 bufs=1)
            nc.vector.tensor_copy(rstd_bf, s2)
            mrep = rep_row(s1_bf)
            rrep = rep_row(rstd_bf)
            hn = []
            for f in range(FT):
                t2 = big.tile([P, TC], F32, tag=f"fhn{f}", name=f"hn{f}")
                nc.vector.tensor_tensor(t2, hc[f], mrep, op=OP.subtract)
                nc.vector.tensor_tensor(t2, t2, rrep, op=OP.mult)
                nc.vector.tensor_scalar(t2, t2, fn_t[f][:, 0:1],
                                        fn_t[f][:, 1:2], OP.mult, OP.add)
                hn.append(t2)
            for tb in range(TC // P):
                ht = small.tile([P, D], BF16, tag="ht", name="ht", bufs=1)
                for f in range(FT):
                    pt = pa.tile([P, TC], F32, tag="mm", name="mm")
                    nc.tensor.transpose(pt[:, 0:P], hn[f][:, tb * P:(tb + 1) * P],
                                        identf)
                    nc.scalar.copy(ht[:, f * P:(f + 1) * P], pt[:, 0:P])
                tglob = t0 + tb * P
                src = ht[:, :]
                rep_in = bass.AP(tensor=src.tensor, offset=src.offset,
                                 ap=[list(src.ap[0]), [0, STRIDE],
                                     list(src.ap[1])])
                dst = y_out[STRIDE * tglob:STRIDE * (tglob + P), :]
                dst3 = dst.rearrange("(t r) d -> t r d", r=STRIDE)
                nc.sync.dma_start(dst3, rep_in)

    nc.compile()
    return nc


# ================================================================ host side
def make_core_inputs(inputs, T=2048, NL=4):
    x = np.asarray(inputs["x"], np.float32)
    conv_w = np.asarray(inputs["conv_w"], np.float32)
    conv_b = np.asarray(inputs["conv_b"], np.float32)
    in_proj_w = np.asarray(inputs["in_proj_w"], np.float32)
    dconv_w = np.asarray(inputs["dconv_w"], np.float32)
    dconv_b = np.asarray(inputs["dconv_b"], np.float32)
    x_proj_w = np.asarray(inputs["x_proj_w"], np.float32)
    dt_proj_w = np.asarray(inputs["dt_proj_w"], np.float32)
    dt_proj_b = np.asarray(inputs["dt_proj_b"], np.float32)
    A_log = np.asarray(inputs["A_log"], np.float32)
    D_param = np.asarray(inputs["D_param"], np.float32)
    out_proj_w = np.asarray(inputs["out_proj_w"], np.float32)
    ln_w = np.asarray(inputs["ln_w"], np.float32)
    ln_b = np.asarray(inputs["ln_b"], np.float32)
    fn_w = np.asarray(inputs["fn_w"], np.float32)
    fn_b = np.asarray(inputs["fn_b"], np.float32)

    Bn = x.shape[0]
    di = x.shape[2]
    dmodel = conv_w.shape[0]
    dinner = in_proj_w.shape[1] // 2
    dh = dinner // 2

    xpad = np.concatenate([np.zeros((Bn, KF - 1, di), np.float32), x], axis=1)
    idx = np.arange(T)[:, None] * STRIDE + np.arange(KF)[None, :]
    xcat = xpad[:, idx, :].reshape(Bn, T, KF * di)
    xcatT = np.ascontiguousarray(xcat.transpose(0, 2, 1))
    wconv = np.ascontiguousarray(conv_w.transpose(2, 1, 0).reshape(KF * di, dmodel))

    A = -np.exp(A_log)

    per_core = []
    for c in range(8):
        b, j = c // 2, c % 2
        sl = slice(j * dh, (j + 1) * dh)
        w_in_l, b_in_l, w_out_l, w_xp_l, wsum_l = [], [], [], [], []
        for l in range(NL):
            Wx = in_proj_w[l, :dinner][sl] * ln_w[l][None, :]
            Wz = in_proj_w[l, dinner:][sl] * ln_w[l][None, :]
            wl = np.concatenate([Wx.T, Wz.T], axis=1)
            w_in_l.append(wl)
            wsum_l.append(-wl.sum(axis=0, keepdims=True))
            bx = in_proj_w[l, :dinner][sl] @ ln_b[l]
            bz = in_proj_w[l, dinner:][sl] @ ln_b[l]
            b_in_l.append(np.concatenate([bx, bz])[:, None])
            w_out_l.append(out_proj_w[l][:, sl].T)
            w_xp_l.append(np.ascontiguousarray(x_proj_w[l][:, sl].T))
        d = dict(
            xcatT=xcatT[b],
            wconv=wconv,
            conv_bias=conv_b[:, None],
            w_in=np.stack(w_in_l),
            b_in=np.stack(b_in_l),
            wsum_neg=np.stack(wsum_l),
            dconv_wt=dconv_w[:, sl, :],
            dconv_bt=dconv_b[:, sl, None],
            w_xp=np.stack(w_xp_l),
            w_dt=np.ascontiguousarray(dt_proj_w[:, sl, :].transpose(0, 2, 1)),
            b_dt=dt_proj_b[:, sl, None],
            a_cols=A[:, sl, :],
            d_par=D_param[:, sl, None],
            w_out=np.stack(w_out_l),
            fn_wb=np.stack([fn_w, fn_b], axis=1),
            identb_bf=np.eye(P, dtype=np.float32),
        )
        per_core.append(d)
    return per_core


def cast_core_inputs(nc, per_core):
    import concourse.mybir as mybir
    want = {}
    for alloc in nc.m.functions[0].allocations:
        if getattr(alloc, "kind", None) == "ExternalInput":
            want[alloc.memorylocations[0].name] = mybir.dt.np(alloc.dtype)
    return [{k: np.ascontiguousarray(np.asarray(v).astype(want[k]))
             for k, v in d.items() if k in want} for d in per_core]


_PROGRAM_CACHE = {}


def get_program(T=2048, NL=4, TC=512):
    key = (T, NL, TC)
    if key not in _PROGRAM_CACHE:
        _PROGRAM_CACHE[key] = build_program(T, NL, TC)
    return _PROGRAM_CACHE[key]


def kernel(**inputs):
    from concourse.bass_utils import run_bass_kernel_spmd
    T = inputs["x"].shape[1] // STRIDE
    NL = inputs["in_proj_w"].shape[0]
    nc = get_program(T, NL)
    per_core = cast_core_inputs(nc, make_core_inputs(inputs, T, NL))
    res = run_bass_kernel_spmd(nc, per_core, core_ids=list(range(8)))
    Bn = inputs["x"].shape[0]
    y = np.stack([res.results[2 * b]["y_out"] for b in range(Bn)])
    return y.astype(np.float32)



# revision 29
# speedup vs baseline: 1.1265x; 1.0232x over previous
"""Trainium2 Bass kernel for nn_ConvBranch: strided-conv front end + 4 Mamba
layers + final LN + x4 upsample.

Sharding (8 cores): core c = (batch b = c//2, d_inner half j = c%2).
Each core: its batch, full sequence T=2048 (post-conv), full d_model=512,
its 512-channel half of d_inner=1024.  Contractions over d_inner (x_proj,
out_proj) produce partial sums -> pair AllReduce ([0,1],[2,3],[4,5],[6,7])
in bf16.

v2 layout/perf notes:
- B/C rows for the scan are broadcast to 128 partitions by stride-0 DMA
  reads from the AllReduce output in DRAM (no PE one-hot matmuls, no
  PSUM->SBUF copies).
- y = sum_n h_n*C_n accumulated on the PE via bf16 identity matmuls into
  PSUM (removes ~1000 DVE/GPSIMD adds).
- LN: stats from a bf16 copy of h; mean folded into in_proj as a rank-1
  correction (host-precomputed -colsum(W)), rstd broadcast via one K=1
  matmul.  All GEMMs bf16 (1 cyc/row).
- Per layer: phase A (LN+in_proj+dconv+x_proj+AllReduce for all chunks)
  then phase B (dt+scan+gate+out_proj+per-chunk y AllReduce) so collective
  latency overlaps compute.
"""

import sys

import numpy as np

sys.path.insert(0, "/opt/trn_rl_repo")

B_ = 4
D_IN = 256
D = 512          # d_model
STRIDE = 4
KF = 8           # front conv kernel
DS = 16          # d_state
DCONV = 4
DI = 1024        # d_inner
DH = DI // 2     # per-core d_inner half
R = 32           # dt_rank
LN_EPS = 1e-5
P = 128
G = DH // P      # 4
FT = D // P      # 4
XP = 64          # x_proj rows: [dt 0:32 | B 32:48 | C 48:64]
GROUPS = [[0, 1], [2, 3], [4, 5], [6, 7]]


# ====================================================================== build
def build_program(T=2048, NL=4, TC=512):
    import contextlib

    import concourse.bacc as bacc
    import concourse.bass as bass
    import concourse.mybir as mybir
    from concourse.tile import TileContext

    F32 = mybir.dt.float32
    BF16 = mybir.dt.bfloat16
    AF = mybir.ActivationFunctionType
    OP = mybir.AluOpType

    TC = min(TC, T)
    NC = T // TC
    assert TC <= 512
    T_IN = T * STRIDE

    nc = bacc.Bacc("TRN2", target_bir_lowering=False, debug=False,
                   enable_asserts=False, num_devices=8)

    xcatT = nc.dram_tensor("xcatT", [2 * STRIDE * D_IN, T], BF16, kind="ExternalInput")
    wconv = nc.dram_tensor("wconv", [2 * STRIDE * D_IN, D], BF16, kind="ExternalInput")
    conv_bias = nc.dram_tensor("conv_bias", [D, 1], F32, kind="ExternalInput")
    w_in = nc.dram_tensor("w_in", [NL, D, 2 * DH], BF16, kind="ExternalInput")
    b_in = nc.dram_tensor("b_in", [NL, 2 * DH, 1], F32, kind="ExternalInput")
    wsum_neg = nc.dram_tensor("wsum_neg", [NL, 1, 2 * DH], BF16, kind="ExternalInput")
    dconv_wt = nc.dram_tensor("dconv_wt", [NL, DH, DCONV], F32, kind="ExternalInput")
    dconv_bt = nc.dram_tensor("dconv_bt", [NL, DH, 1], F32, kind="ExternalInput")
    w_xp = nc.dram_tensor("w_xp", [NL, DH, XP], BF16, kind="ExternalInput")
    w_dt = nc.dram_tensor("w_dt", [NL, R, DH], BF16, kind="ExternalInput")
    b_dt = nc.dram_tensor("b_dt", [NL, DH, 1], F32, kind="ExternalInput")
    a_cols = nc.dram_tensor("a_cols", [NL, DH, DS], F32, kind="ExternalInput")
    d_par = nc.dram_tensor("d_par", [NL, DH, 1], F32, kind="ExternalInput")
    w_out = nc.dram_tensor("w_out", [NL, DH, D], BF16, kind="ExternalInput")
    fn_wb = nc.dram_tensor("fn_wb", [D, 2], F32, kind="ExternalInput")
    identb_bf = nc.dram_tensor("identb_bf", [P, P], BF16, kind="ExternalInput")
    y_out = nc.dram_tensor("y_out", [T_IN, D], BF16, kind="ExternalOutput")

    NCK = NL * NC
    cc_dbc_i = [nc.dram_tensor(f"cc_dbc_i{k}", [XP, T // 2], BF16) for k in range(2 * NL)]
    cc_dbc_o = [nc.dram_tensor(f"cc_dbc_o{k}", [XP, T // 2], BF16) for k in range(2 * NL)]
    cc_y_i = [nc.dram_tensor(f"cc_y_i{k}", [D, TC], BF16) for k in range(NCK)]
    cc_y_o = [nc.dram_tensor(f"cc_y_o{k}", [D, TC], BF16) for k in range(NCK)]

    def bcast_rows(dram_rows):
        """[R, W] DRAM rows -> stride-0 AP readable as [P, R, W]."""
        return bass.AP(tensor=dram_rows.tensor, offset=dram_rows.offset,
                       ap=[[0, P]] + [list(d) for d in dram_rows.ap])

    with TileContext(nc) as tc, contextlib.ExitStack() as ctx:
        persist = ctx.enter_context(tc.tile_pool(name="persist", bufs=1))
        wpool = ctx.enter_context(tc.tile_pool(name="wpool", bufs=1))
        big = ctx.enter_context(tc.tile_pool(name="big", bufs=1))
        scanp = ctx.enter_context(tc.tile_pool(name="scanp", bufs=2))
        bc = ctx.enter_context(tc.tile_pool(name="bc", bufs=1))
        small = ctx.enter_context(tc.tile_pool(name="small", bufs=2))

        ones_col_bf = persist.tile([P, 1], BF16)
        nc.vector.memset(ones_col_bf, 1.0 / D)
        ones_row_bf = persist.tile([1, P], BF16)
        nc.vector.memset(ones_row_bf, 1.0)
        identb = persist.tile([P, P], BF16)
        nc.sync.dma_start(identb, identb_bf[:, :])
        identf = persist.tile([P, P], F32)
        nc.scalar.copy(identf, identb)
        eps_t = persist.tile([P, 1], F32)
        nc.vector.memset(eps_t, LN_EPS)

        h = [persist.tile([P, T], F32, name=f"h{f}") for f in range(FT)]

        # ------------------------------------------------- front conv + GELU
        with tc.tile_pool(name="convp", bufs=1) as convp, \
             tc.tile_pool(name="convx", bufs=4) as convx, \
             tc.tile_pool(name="convps", bufs=4, space="PSUM") as convps:
            K16 = (2 * STRIDE * D_IN) // P
            cb = []
            for f in range(FT):
                cbf = convp.tile([P, 1], F32, name=f"cb{f}")
                nc.sync.dma_start(cbf, conv_bias[f * P:(f + 1) * P, :])
                cb.append(cbf)
            for c in range(T // TC):
                pts = [convps.tile([P, TC], F32, tag="mm", name="mm")
                       for _ in range(FT)]
                for k in range(K16):
                    wt = convx.tile([P, D], BF16, tag="wc", name="wc", bufs=2)
                    nc.sync.dma_start(wt, wconv[k * P:(k + 1) * P, :])
                    xt = convx.tile([P, TC], BF16, tag="xcat", name="xcat", bufs=2)
                    nc.sync.dma_start(xt, xcatT[k * P:(k + 1) * P,
                                                c * TC:(c + 1) * TC])
                    for f in range(FT):
                        nc.tensor.matmul(pts[f], wt[:, f * P:(f + 1) * P],
                                         xt, start=(k == 0), stop=(k == K16 - 1))
                for f in range(FT):
                    nc.scalar.activation(h[f][:, c * TC:(c + 1) * TC], pts[f],
                                         AF.Gelu, bias=cb[f], scale=1.0)

        pa = ctx.enter_context(tc.tile_pool(name="pa", bufs=2, space="PSUM"))
        pb = ctx.enter_context(tc.tile_pool(name="pb", bufs=2, space="PSUM"))
        pyac = ctx.enter_context(tc.tile_pool(name="pyac", bufs=1, space="PSUM"))

        def ln_stats(sl, ci=0):
            """bf16 copy of h chunk + mean/rstd rows; returns (hc, s1, rstd)."""
            stat = pa.tile([P, TC], F32, tag="mm", name="stat")
            hc = []
            for f in range(FT):
                c = big.tile([P, TC], BF16, tag=f"hc{f}_{ci}", name=f"hc{f}")
                nc.scalar.copy(c, h[f][:, sl])
                hc.append(c)
                nc.tensor.matmul(stat[0:1, :], ones_col_bf, c,
                                 start=(f == 0), stop=(f == FT - 1))
            for f in range(FT):
                q = big.tile([P, TC], BF16, tag="hsq", name="hsq", bufs=1)
                nc.scalar.activation(q, hc[f], AF.Square)
                nc.tensor.matmul(stat[32:33, :], ones_col_bf, q,
                                 start=(f == 0), stop=(f == FT - 1))
            s1 = small.tile([1, TC], F32, tag="s1", name="s1", bufs=1)
            nc.scalar.copy(s1, stat[0:1, :])               # mean (ones = 1/D)
            msq = small.tile([1, TC], F32, tag="msq", name="msq", bufs=1)
            nc.scalar.activation(msq, s1, AF.Square)
            s2 = small.tile([1, TC], F32, tag="s2", name="s2", bufs=1)
            nc.vector.tensor_tensor(s2, stat[32:33, :], msq, op=OP.subtract)
            nc.scalar.activation(s2, s2, AF.Ln, bias=eps_t[0:1, :], scale=1.0)
            nc.scalar.activation(s2, s2, AF.Exp, scale=-0.5)  # rstd
            return hc, s1, s2

        def rep_row(row_bf):
            """Broadcast a [1, TC] bf16 row to a [P, TC] bf16 tile via PE."""
            rp = pa.tile([P, TC], F32, tag="mm", name="rep")
            nc.tensor.matmul(rp, ones_row_bf, row_bf, start=True, stop=True)
            out = big.tile([P, TC], BF16, tag="rrep", name="rrep", bufs=4)
            nc.scalar.copy(out, rp)
            return out

        # ---------------------------------------------------------- layers
        for l in range(NL):
            w_in_t = [wpool.tile([P, 2 * DH], BF16, tag=f"w_in{k}",
                                 name=f"w_in{k}") for k in range(FT)]
            for k in range(FT):
                nc.sync.dma_start(w_in_t[k], w_in[l, k * P:(k + 1) * P, :])
            wsum_t = wpool.tile([1, 2 * DH], BF16, tag="wsum", name="wsum")
            nc.sync.dma_start(wsum_t, wsum_neg[l])
            b_in_t = [wpool.tile([P, 1], F32, tag=f"b_in{e}", name=f"b_in{e}")
                      for e in range(2 * DH // P)]
            for e in range(2 * DH // P):
                nc.sync.dma_start(b_in_t[e], b_in[l, e * P:(e + 1) * P, :])
            dcw_t = [wpool.tile([P, DCONV], F32, tag=f"dcw{g}", name=f"dcw{g}")
                     for g in range(G)]
            dcb_t = [wpool.tile([P, 1], F32, tag=f"dcb{g}", name=f"dcb{g}")
                     for g in range(G)]
            w_xp_t = [wpool.tile([P, XP], BF16, tag=f"w_xp{g}", name=f"w_xp{g}")
                      for g in range(G)]
            b_dt_t = [wpool.tile([P, 1], F32, tag=f"b_dt{g}", name=f"b_dt{g}")
                      for g in range(G)]
            ac_t = [wpool.tile([P, DS], F32, tag=f"ac{g}", name=f"ac{g}")
                    for g in range(G)]
            dpar_t = [wpool.tile([P, 1], F32, tag=f"dpar{g}", name=f"dpar{g}")
                      for g in range(G)]
            w_out_t = [wpool.tile([P, D], BF16, tag=f"w_out{g}", name=f"w_out{g}")
                       for g in range(G)]
            for g in range(G):
                s = slice(g * P, (g + 1) * P)
                nc.sync.dma_start(dcw_t[g], dconv_wt[l, s, :])
                nc.sync.dma_start(dcb_t[g], dconv_bt[l, s, :])
                nc.sync.dma_start(w_xp_t[g], w_xp[l, s, :])
                nc.sync.dma_start(b_dt_t[g], b_dt[l, s, :])
                nc.sync.dma_start(ac_t[g], a_cols[l, s, :])
                nc.sync.dma_start(dpar_t[g], d_par[l, s, :])
                nc.sync.dma_start(w_out_t[g], w_out[l, s, :])
            w_dt_t = wpool.tile([R, DH], BF16, tag="w_dt", name="w_dt")
            nc.sync.dma_start(w_dt_t, w_dt[l])

            # full-T concat buffers (xs written in-place over xraw after dconv)
            # pad to 4 so the xs view starts at an even element offset (DVE 2x)
            PAD = 4
            xraw_c = [big.tile([P, PAD + T], BF16, tag=f"xrc{g}",
                               name=f"xrc{g}") for g in range(G)]
            zs_c = [big.tile([P, T], BF16, tag=f"zsc{g}", name=f"zsc{g}")
                    for g in range(G)]
            for g in range(G):
                nc.vector.memset(xraw_c[g][:, 0:PAD], 0.0)

            # ===== phase A: stats for all chunks first (cross-chunk pipelining)
            srows = []
            for ci in range(NC):
                t0 = ci * TC
                hc, s1, s2 = ln_stats(slice(t0, t0 + TC), ci)
                rstd_bf = small.tile([1, TC], BF16, tag="rb", name="rb", bufs=1)
                nc.scalar.copy(rstd_bf, s2)
                mr_row = small.tile([1, TC], BF16, tag=f"mr{ci}", name="mr",
                                    bufs=1)
                nc.vector.tensor_tensor(mr_row, s1, s2, op=OP.mult)
                srows.append((hc, mr_row, rep_row(rstd_bf)))
            # ===== in_proj per chunk into concat buffers
            for ci in range(NC):
                t0 = ci * TC
                sl = slice(t0, t0 + TC)
                hc, mr_row, rrep = srows[ci]
                hcn = hc
                for f in range(FT):
                    nc.vector.tensor_tensor(hcn[f], hc[f], rrep, op=OP.mult)
                for half in range(2):
                    for g in range(G):
                        e0 = half * DH + g * P
                        pt = pa.tile([P, TC], F32, tag="mm", name="mm")
                        for k in range(FT):
                            nc.tensor.matmul(pt, w_in_t[k][:, e0:e0 + P],
                                             hcn[k], start=(k == 0), stop=False)
                        nc.tensor.matmul(pt, wsum_t[:, e0:e0 + P], mr_row,
                                         start=False, stop=True)
                        bia = b_in_t[e0 // P]
                        if half == 0:
                            nc.scalar.activation(
                                xraw_c[g][:, PAD + t0:PAD + t0 + TC],
                                pt, AF.Identity, bias=bia, scale=1.0)
                        else:
                            nc.scalar.activation(zs_c[g][:, sl], pt, AF.Silu,
                                                 bias=bia, scale=1.0)

            # ============ dconv on gpsimd, per T-half (in-place silu -> xs)
            TH = T // 2
            xs_c = []
            for g in range(G):
                for hh in range(2):
                    o0 = hh * TH
                    tmp = small.tile([P, TH], BF16, tag="dctmp", name="dctmp",
                                     bufs=2)
                    nc.vector.tensor_scalar(tmp, xraw_c[g][:, 1 + o0:1 + o0 + TH],
                                            dcw_t[g][:, 0:1], None, OP.mult)
                    for j in range(1, DCONV):
                        tmp2 = small.tile([P, TH], BF16, tag="dctmp",
                                          name="dctmp", bufs=2)
                        nc.vector.scalar_tensor_tensor(
                            tmp2, xraw_c[g][:, 1 + j + o0:1 + j + o0 + TH],
                            dcw_t[g][:, j:j + 1], tmp, op0=OP.mult, op1=OP.add)
                        tmp = tmp2
                    nc.scalar.activation(xraw_c[g][:, PAD + o0:PAD + o0 + TH],
                                         tmp, AF.Silu, bias=dcb_t[g],
                                         scale=1.0)
                xs_c.append(xraw_c[g][:, PAD:PAD + T])

            # ===== x_proj partials + pair AllReduce per T-half (bf16)
            T2 = T // 2
            for hf in range(2):
                for cj in range(2):
                    t0 = (2 * hf + cj) * TC
                    pt = pa.tile([P, TC], F32, tag="mm", name="mm")
                    for g in range(G):
                        nc.tensor.matmul(pt[0:XP, :], w_xp_t[g],
                                         xs_c[g][:, t0:t0 + TC],
                                         start=(g == 0), stop=(g == G - 1))
                    dbc_p = small.tile([XP, TC], BF16, tag="dbc_p",
                                       name="dbc_p", bufs=1)
                    nc.scalar.copy(dbc_p, pt[0:XP, :])
                    nc.sync.dma_start(
                        cc_dbc_i[2 * l + hf][:, cj * TC:(cj + 1) * TC], dbc_p)
                nc.gpsimd.collective_compute(
                    "AllReduce", OP.add, replica_groups=GROUPS,
                    ins=[cc_dbc_i[2 * l + hf][:, :]],
                    outs=[cc_dbc_o[2 * l + hf][:, :]])

            dt_c = [big.tile([P, T], BF16, tag=f"dtc{g}", name=f"dtc{g}")
                    for g in range(G)]
            dtu_c = [big.tile([P, T], BF16, tag=f"dtuc{g}", name=f"dtuc{g}")
                     for g in range(G)]
            fin = [big.tile([P, DS], F32, tag=f"fin{g}", name=f"fin{g}")
                   for g in range(G)]

            def emit_dt(hf):
                """dt = softplus(w_dt @ dbc_dt + b); dtu = dt*xs (one T-half)."""
                h0 = hf * T2
                hsl = slice(h0, h0 + T2)
                cc = cc_dbc_o[2 * l + hf]
                dbc_dt = small.tile([R, T2], BF16, tag="dbc_dt",
                                    name="dbc_dt", bufs=1)
                nc.sync.dma_start(dbc_dt, cc[0:R, :])
                # softplus: all exps first, then all lns in-place (1 table swap)
                for g in range(G):
                    for cj in range(2):
                        ptd = pb.tile([P, TC], F32, tag="mm", name="mm")
                        nc.tensor.matmul(ptd, w_dt_t[:, g * P:(g + 1) * P],
                                         dbc_dt[:, cj * TC:(cj + 1) * TC],
                                         start=True, stop=True)
                        nc.scalar.activation(
                            dt_c[g][:, h0 + cj * TC:h0 + (cj + 1) * TC], ptd,
                            AF.Exp, bias=b_dt_t[g], scale=1.0)
                for g in range(G):
                    for cj in range(2):
                        sl2 = slice(h0 + cj * TC, h0 + (cj + 1) * TC)
                        nc.scalar.activation(dt_c[g][:, sl2], dt_c[g][:, sl2],
                                             AF.Ln, bias=1.0, scale=1.0)
                for g in range(G):
                    nc.vector.tensor_tensor(dtu_c[g][:, hsl], dt_c[g][:, hsl],
                                            xs_c[g][:, hsl], op=OP.mult)

            emit_dt(0)
            for hf in range(2):
                h0 = hf * T2
                hsl = slice(h0, h0 + T2)
                cc = cc_dbc_o[2 * l + hf]
                # ---- scan blocks: g-pairs share B/C fetches
                for gp in ((0, 1), (2, 3)):
                    # overlap: emit next half's dt work mid-way so Act/PE
                    # compute it during this half's scans
                    if hf == 0 and gp == (2, 3):
                        emit_dt(1)
                    ypgs = {}
                    for g in gp:
                        yp = pyac.tile([P, T2], F32, tag=f"y{g % 2}",
                                       name=f"y{g % 2}")
                        for cj in range(2):
                            xsD = small.tile([P, TC], BF16, tag="xsD",
                                             name="xsD", bufs=1)
                            nc.vector.tensor_scalar(
                                xsD, xs_c[g][:, h0 + cj * TC:h0 + (cj + 1) * TC],
                                dpar_t[g], None, OP.mult)
                            nc.tensor.matmul(yp[:, cj * TC:(cj + 1) * TC],
                                             identb, xsD, start=True,
                                             stop=False)
                        ypgs[g] = yp

                    def fetch_q(q):
                        """One DMA per 2-row pair of B rows / C rows."""
                        bt = bc.tile([P, 2 * T2], BF16, tag=f"bq{q % 2}",
                                     name=f"bq{q % 2}")
                        ct = bc.tile([P, 2 * T2], BF16, tag=f"cq{q % 2}",
                                     name=f"cq{q % 2}")
                        bdst = bass.AP(tensor=bt.tensor, offset=bt.offset,
                                       ap=[list(bt.ap[0]), [T2, 2], [1, T2]])
                        cdst = bass.AP(tensor=ct.tensor, offset=ct.offset,
                                       ap=[list(ct.ap[0]), [T2, 2], [1, T2]])
                        nc.scalar.dma_start(
                            bdst, bcast_rows(cc[R + 2 * q:R + 2 * q + 2, :]),
                            single_packet=True)
                        nc.sync.dma_start(
                            cdst,
                            bcast_rows(cc[R + DS + 2 * q:R + DS + 2 * q + 2, :]),
                            single_packet=True)
                        return bt, ct

                    bcache = {0: fetch_q(0), 1: fetch_q(1)}
                    for q in range(8):
                        bt_q, ct_q = bcache[q % 2]
                        for ni in range(2):
                            if ni == 1 and 0 < q < 7:
                                # prefetch q+1 here so the DMA transfer
                                # overlaps ni=1's scans, not the cm/b_t ops
                                bcache[(q + 1) % 2] = fetch_q(q + 1)
                            n = 2 * q + ni
                            brep = bt_q[:, ni * T2:(ni + 1) * T2]
                            crep = ct_q[:, ni * T2:(ni + 1) * T2]
                            hscs = {}
                            for g in gp:
                                a_t = scanp.tile([P, T2], BF16, tag="sa",
                                                 name="sa", bufs=2)
                                nc.scalar.activation(a_t, dt_c[g][:, hsl],
                                                     AF.Exp,
                                                     scale=ac_t[g][:, n:n + 1])
                                b_t = scanp.tile([P, T2], BF16, tag="sb",
                                                 name="sb")
                                nc.vector.tensor_tensor(b_t, dtu_c[g][:, hsl],
                                                        brep, op=OP.mult)
                                hsc = scanp.tile([P, T2], BF16, tag="sh",
                                                 name="sh")
                                init = 0.0 if hf == 0 else fin[g][:, n:n + 1]
                                nc.vector.tensor_tensor_scan(hsc, a_t, b_t,
                                                             init,
                                                             op0=OP.mult,
                                                             op1=OP.add)
                                if hf == 0:
                                    nc.vector.tensor_copy(fin[g][:, n:n + 1],
                                                          hsc[:, T2 - 1:T2])
                                hscs[g] = hsc
                            # cm after both scans (avoid read-after-write
                            # stall on hsc); one per engine per n
                            for gi, g in enumerate(gp):
                                cm = scanp.tile([P, T2], BF16, tag="sm",
                                                name="sm")
                                nc.vector.tensor_tensor(cm, hscs[g], crep,
                                                        op=OP.mult)
                                for cj in range(2):
                                    nc.tensor.matmul(
                                        ypgs[g][:, cj * TC:(cj + 1) * TC],
                                        identb, cm[:, cj * TC:(cj + 1) * TC],
                                        start=False, stop=(n == DS - 1))
                    # gating into dtu_c (dead after n loop)
                    for g in gp:
                        nc.vector.tensor_tensor(dtu_c[g][:, hsl], ypgs[g],
                                                zs_c[g][:, hsl], op=OP.mult)

                # ---- out_proj + AR_y + residual for this half's chunks
                for cj in range(2):
                    ci = 2 * hf + cj
                    t0 = ci * TC
                    sl = slice(t0, t0 + TC)
                    ccidx = l * NC + ci
                    for f in range(FT):
                        po = pb.tile([P, TC], F32, tag="mm", name="mm")
                        for g in range(G):
                            nc.tensor.matmul(po,
                                             w_out_t[g][:, f * P:(f + 1) * P],
                                             dtu_c[g][:, sl], start=(g == 0),
                                             stop=(g == G - 1))
                        ot = small.tile([P, TC], BF16, tag="oout", name="oout")
                        nc.scalar.copy(ot, po)
                        nc.sync.dma_start(cc_y_i[ccidx][f * P:(f + 1) * P, :],
                                          ot)
                    nc.gpsimd.collective_compute(
                        "AllReduce", OP.add, replica_groups=GROUPS,
                        ins=[cc_y_i[ccidx][:, :]], outs=[cc_y_o[ccidx][:, :]])
                    for f in range(FT):
                        yfull = scanp.tile([P, TC], BF16, tag="yfull",
                                           name="yfull", bufs=2)
                        nc.sync.dma_start(yfull,
                                          cc_y_o[ccidx][f * P:(f + 1) * P, :])
                        nc.vector.tensor_tensor(h[f][:, sl], h[f][:, sl],
                                                yfull, op=OP.add)

        # ------------------------------------------- final LN + transpose out
        fn_t = [persist.tile([P, 2], F32, name=f"fn{f}") for f in range(FT)]
        for f in range(FT):
            nc.sync.dma_start(fn_t[f], fn_wb[f * P:(f + 1) * P, :])
        for ci in range(NC):
            t0 = ci * TC
            sl = slice(t0, t0 + TC)
            hc, s1, s2 = ln_stats(sl, ci)
            s1_bf = small.tile([1, TC], BF16, tag=f"mr{ci}", name="mr", bufs=1)
            nc.vector.tensor_copy(s1_bf, s1)
            rstd_bf = small.tile([1, TC], BF16, tag=f"rb{ci}", name="rb",
                                 bufs=1)
            nc.vector.tensor_copy(rstd_bf, s2)
            mrep = rep_row(s1_bf)
            rrep = rep_row(rstd_bf)
            hn = []
            for f in range(FT):
                t2 = big.tile([P, TC], F32, tag=f"fhn{f}", name=f"hn{f}")
                nc.vector.tensor_tensor(t2, hc[f], mrep, op=OP.subtract)
                nc.vector.tensor_tensor(t2, t2, rrep, op=OP.mult)
                nc.vector.tensor_scalar(t2, t2, fn_t[f][:, 0:1],
                                        fn_t[f][:, 1:2], OP.mult, OP.add)
                hn.append(t2)
            for tb in range(TC // P):
                ht = small.tile([P, D], BF16, tag="ht", name="ht", bufs=1)
                for f in range(FT):
                    pt = pa.tile([P, TC], F32, tag="mm", name="mm")
                    nc.tensor.transpose(pt[:, 0:P], hn[f][:, tb * P:(tb + 1) * P],
                                        identf)
                    nc.scalar.copy(ht[:, f * P:(f + 1) * P], pt[:, 0:P])
                tglob = t0 + tb * P
                src = ht[:, :]
                rep_in = bass.AP(tensor=src.tensor, offset=src.offset,
                                 ap=[list(src.ap[0]), [0, STRIDE],
                                     list(src.ap[1])])
                dst = y_out[STRIDE * tglob:STRIDE * (tglob + P), :]
                dst3 = dst.rearrange("(t r) d -> t r d", r=STRIDE)
                nc.sync.dma_start(dst3, rep_in)

    nc.compile()
    return nc


# ================================================================ host side
def make_core_inputs(inputs, T=2048, NL=4):
    x = np.asarray(inputs["x"], np.float32)
    conv_w = np.asarray(inputs["conv_w"], np.float32)
    conv_b = np.asarray(inputs["conv_b"], np.float32)
    in_proj_w = np.asarray(inputs["in_proj_w"], np.float32)
    dconv_w = np.asarray(inputs["dconv_w"], np.float32)
    dconv_b = np.asarray(inputs["dconv_b"], np.float32)
    x_proj_w = np.asarray(inputs["x_proj_w"], np.float32)
    dt_proj_w = np.asarray(inputs["dt_proj_w"], np.float32)
    dt_proj_b = np.asarray(inputs["dt_proj_b"], np.float32)
    A_log = np.asarray(inputs["A_log"], np.float32)
    D_param = np.asarray(inputs["D_param"], np.float32)
    out_proj_w = np.asarray(inputs["out_proj_w"], np.float32)
    ln_w = np.asarray(inputs["ln_w"], np.float32)
    ln_b = np.asarray(inputs["ln_b"], np.float32)
    fn_w = np.asarray(inputs["fn_w"], np.float32)
    fn_b = np.asarray(inputs["fn_b"], np.float32)

    Bn = x.shape[0]
    di = x.shape[2]
    dmodel = conv_w.shape[0]
    dinner = in_proj_w.shape[1] // 2
    dh = dinner // 2

    xpad = np.concatenate([np.zeros((Bn, KF - 1, di), np.float32), x], axis=1)
    idx = np.arange(T)[:, None] * STRIDE + np.arange(KF)[None, :]
    xcat = xpad[:, idx, :].reshape(Bn, T, KF * di)
    xcatT = np.ascontiguousarray(xcat.transpose(0, 2, 1))
    wconv = np.ascontiguousarray(conv_w.transpose(2, 1, 0).reshape(KF * di, dmodel))

    A = -np.exp(A_log)

    per_core = []
    for c in range(8):
        b, j = c // 2, c % 2
        sl = slice(j * dh, (j + 1) * dh)
        w_in_l, b_in_l, w_out_l, w_xp_l, wsum_l = [], [], [], [], []
        for l in range(NL):
            Wx = in_proj_w[l, :dinner][sl] * ln_w[l][None, :]
            Wz = in_proj_w[l, dinner:][sl] * ln_w[l][None, :]
            wl = np.concatenate([Wx.T, Wz.T], axis=1)
            w_in_l.append(wl)
            wsum_l.append(-wl.sum(axis=0, keepdims=True))
            bx = in_proj_w[l, :dinner][sl] @ ln_b[l]
            bz = in_proj_w[l, dinner:][sl] @ ln_b[l]
            b_in_l.append(np.concatenate([bx, bz])[:, None])
            w_out_l.append(out_proj_w[l][:, sl].T)
            w_xp_l.append(np.ascontiguousarray(x_proj_w[l][:, sl].T))
        d = dict(
            xcatT=xcatT[b],
            wconv=wconv,
            conv_bias=conv_b[:, None],
            w_in=np.stack(w_in_l),
            b_in=np.stack(b_in_l),
            wsum_neg=np.stack(wsum_l),
            dconv_wt=dconv_w[:, sl, :],
            dconv_bt=dconv_b[:, sl, None],
            w_xp=np.stack(w_xp_l),
            w_dt=np.ascontiguousarray(dt_proj_w[:, sl, :].transpose(0, 2, 1)),
            b_dt=dt_proj_b[:, sl, None],
            a_cols=A[:, sl, :],
            d_par=D_param[:, sl, None],
            w_out=np.stack(w_out_l),
            fn_wb=np.stack([fn_w, fn_b], axis=1),
            identb_bf=np.eye(P, dtype=np.float32),
        )
        per_core.append(d)
    return per_core


def cast_core_inputs(nc, per_core):
    import concourse.mybir as mybir
    want = {}
    for alloc in nc.m.functions[0].allocations:
        if getattr(alloc, "kind", None) == "ExternalInput":
            want[alloc.memorylocations[0].name] = mybir.dt.np(alloc.dtype)
    return [{k: np.ascontiguousarray(np.asarray(v).astype(want[k]))
             for k, v in d.items() if k in want} for d in per_core]


_PROGRAM_CACHE = {}


def get_program(T=2048, NL=4, TC=512):
    key = (T, NL, TC)
    if key not in _PROGRAM_CACHE:
        _PROGRAM_CACHE[key] = build_program(T, NL, TC)
    return _PROGRAM_CACHE[key]


def kernel(**inputs):
    from concourse.bass_utils import run_bass_kernel_spmd
    T = inputs["x"].shape[1] // STRIDE
    NL = inputs["in_proj_w"].shape[0]
    nc = get_program(T, NL)
    per_core = cast_core_inputs(nc, make_core_inputs(inputs, T, NL))
    res = run_bass_kernel_spmd(nc, per_core, core_ids=list(range(8)))
    Bn = inputs["x"].shape[0]
    y = np.stack([res.results[2 * b]["y_out"] for b in range(Bn)])
    return y.astype(np.float32)



# revision 30
# speedup vs baseline: 1.1381x; 1.0102x over previous
"""Trainium2 Bass kernel for nn_ConvBranch: strided-conv front end + 4 Mamba
layers + final LN + x4 upsample.

Sharding (8 cores): core c = (batch b = c//2, d_inner half j = c%2).
Each core: its batch, full sequence T=2048 (post-conv), full d_model=512,
its 512-channel half of d_inner=1024.  Contractions over d_inner (x_proj,
out_proj) produce partial sums -> pair AllReduce ([0,1],[2,3],[4,5],[6,7])
in bf16.

v2 layout/perf notes:
- B/C rows for the scan are broadcast to 128 partitions by stride-0 DMA
  reads from the AllReduce output in DRAM (no PE one-hot matmuls, no
  PSUM->SBUF copies).
- y = sum_n h_n*C_n accumulated on the PE via bf16 identity matmuls into
  PSUM (removes ~1000 DVE/GPSIMD adds).
- LN: stats from a bf16 copy of h; mean folded into in_proj as a rank-1
  correction (host-precomputed -colsum(W)), rstd broadcast via one K=1
  matmul.  All GEMMs bf16 (1 cyc/row).
- Per layer: phase A (LN+in_proj+dconv+x_proj+AllReduce for all chunks)
  then phase B (dt+scan+gate+out_proj+per-chunk y AllReduce) so collective
  latency overlaps compute.
"""

import sys

import numpy as np

sys.path.insert(0, "/opt/trn_rl_repo")

B_ = 4
D_IN = 256
D = 512          # d_model
STRIDE = 4
KF = 8           # front conv kernel
DS = 16          # d_state
DCONV = 4
DI = 1024        # d_inner
DH = DI // 2     # per-core d_inner half
R = 32           # dt_rank
LN_EPS = 1e-5
P = 128
G = DH // P      # 4
FT = D // P      # 4
XP = 64          # x_proj rows: [dt 0:32 | B 32:48 | C 48:64]
GROUPS = [[0, 1], [2, 3], [4, 5], [6, 7]]


# ====================================================================== build
def build_program(T=2048, NL=4, TC=512):
    import contextlib

    import concourse.bacc as bacc
    import concourse.bass as bass
    import concourse.mybir as mybir
    from concourse.tile import TileContext

    F32 = mybir.dt.float32
    BF16 = mybir.dt.bfloat16
    AF = mybir.ActivationFunctionType
    OP = mybir.AluOpType

    TC = min(TC, T)
    NC = T // TC
    assert TC <= 512
    T_IN = T * STRIDE

    nc = bacc.Bacc("TRN2", target_bir_lowering=False, debug=False,
                   enable_asserts=False, num_devices=8)

    xcatT = nc.dram_tensor("xcatT", [2 * STRIDE * D_IN, T], BF16, kind="ExternalInput")
    wconv = nc.dram_tensor("wconv", [2 * STRIDE * D_IN, D], BF16, kind="ExternalInput")
    conv_bias = nc.dram_tensor("conv_bias", [D, 1], F32, kind="ExternalInput")
    w_in = nc.dram_tensor("w_in", [NL, D, 2 * DH], BF16, kind="ExternalInput")
    b_in = nc.dram_tensor("b_in", [NL, 2 * DH, 1], F32, kind="ExternalInput")
    wsum_neg = nc.dram_tensor("wsum_neg", [NL, 1, 2 * DH], BF16, kind="ExternalInput")
    dconv_wt = nc.dram_tensor("dconv_wt", [NL, DH, DCONV], F32, kind="ExternalInput")
    dconv_bt = nc.dram_tensor("dconv_bt", [NL, DH, 1], F32, kind="ExternalInput")
    w_xp = nc.dram_tensor("w_xp", [NL, DH, XP], BF16, kind="ExternalInput")
    w_dt = nc.dram_tensor("w_dt", [NL, R, DH], BF16, kind="ExternalInput")
    b_dt = nc.dram_tensor("b_dt", [NL, DH, 1], F32, kind="ExternalInput")
    a_cols = nc.dram_tensor("a_cols", [NL, DH, DS], F32, kind="ExternalInput")
    d_par = nc.dram_tensor("d_par", [NL, DH, 1], F32, kind="ExternalInput")
    w_out = nc.dram_tensor("w_out", [NL, DH, D], BF16, kind="ExternalInput")
    fn_wb = nc.dram_tensor("fn_wb", [D, 2], F32, kind="ExternalInput")
    identb_bf = nc.dram_tensor("identb_bf", [P, P], BF16, kind="ExternalInput")
    y_out = nc.dram_tensor("y_out", [T_IN, D], BF16, kind="ExternalOutput")

    NCK = NL * NC
    cc_dbc_i = [nc.dram_tensor(f"cc_dbc_i{k}", [XP, T // 2], BF16) for k in range(2 * NL)]
    cc_dbc_o = [nc.dram_tensor(f"cc_dbc_o{k}", [XP, T // 2], BF16) for k in range(2 * NL)]
    cc_y_i = [nc.dram_tensor(f"cc_y_i{k}", [D, TC], BF16) for k in range(NCK)]
    cc_y_o = [nc.dram_tensor(f"cc_y_o{k}", [D, TC], BF16) for k in range(NCK)]

    def bcast_rows(dram_rows):
        """[R, W] DRAM rows -> stride-0 AP readable as [P, R, W]."""
        return bass.AP(tensor=dram_rows.tensor, offset=dram_rows.offset,
                       ap=[[0, P]] + [list(d) for d in dram_rows.ap])

    with TileContext(nc) as tc, contextlib.ExitStack() as ctx:
        persist = ctx.enter_context(tc.tile_pool(name="persist", bufs=1))
        wpool = ctx.enter_context(tc.tile_pool(name="wpool", bufs=1))
        big = ctx.enter_context(tc.tile_pool(name="big", bufs=1))
        scanp = ctx.enter_context(tc.tile_pool(name="scanp", bufs=2))
        bc = ctx.enter_context(tc.tile_pool(name="bc", bufs=1))
        small = ctx.enter_context(tc.tile_pool(name="small", bufs=2))

        ones_col_bf = persist.tile([P, 1], BF16)
        nc.vector.memset(ones_col_bf, 1.0 / D)
        ones_row_bf = persist.tile([1, P], BF16)
        nc.vector.memset(ones_row_bf, 1.0)
        identb = persist.tile([P, P], BF16)
        nc.sync.dma_start(identb, identb_bf[:, :])
        identf = persist.tile([P, P], F32)
        nc.scalar.copy(identf, identb)
        eps_t = persist.tile([P, 1], F32)
        nc.vector.memset(eps_t, LN_EPS)

        h = [persist.tile([P, T], F32, name=f"h{f}") for f in range(FT)]

        # ------------------------------------------------- front conv + GELU
        with tc.tile_pool(name="convp", bufs=1) as convp, \
             tc.tile_pool(name="convx", bufs=4) as convx, \
             tc.tile_pool(name="convps", bufs=4, space="PSUM") as convps:
            K16 = (2 * STRIDE * D_IN) // P
            cb = []
            for f in range(FT):
                cbf = convp.tile([P, 1], F32, name=f"cb{f}")
                nc.sync.dma_start(cbf, conv_bias[f * P:(f + 1) * P, :])
                cb.append(cbf)
            for c in range(T // TC):
                pts = [convps.tile([P, TC], F32, tag="mm", name="mm")
                       for _ in range(FT)]
                for k in range(K16):
                    wt = convx.tile([P, D], BF16, tag="wc", name="wc", bufs=2)
                    nc.sync.dma_start(wt, wconv[k * P:(k + 1) * P, :])
                    xt = convx.tile([P, TC], BF16, tag="xcat", name="xcat", bufs=2)
                    nc.sync.dma_start(xt, xcatT[k * P:(k + 1) * P,
                                                c * TC:(c + 1) * TC])
                    for f in range(FT):
                        nc.tensor.matmul(pts[f], wt[:, f * P:(f + 1) * P],
                                         xt, start=(k == 0), stop=(k == K16 - 1))
                for f in range(FT):
                    nc.scalar.activation(h[f][:, c * TC:(c + 1) * TC], pts[f],
                                         AF.Gelu, bias=cb[f], scale=1.0)

        pa = ctx.enter_context(tc.tile_pool(name="pa", bufs=2, space="PSUM"))
        pb = ctx.enter_context(tc.tile_pool(name="pb", bufs=2, space="PSUM"))
        pyac = ctx.enter_context(tc.tile_pool(name="pyac", bufs=1, space="PSUM"))

        def ln_stats(sl, ci=0):
            """bf16 copy of h chunk + mean/rstd rows; returns (hc, s1, rstd)."""
            stat = pa.tile([P, TC], F32, tag="mm", name="stat")
            hc = []
            for f in range(FT):
                c = big.tile([P, TC], BF16, tag=f"hc{f}_{ci}", name=f"hc{f}")
                nc.scalar.copy(c, h[f][:, sl])
                hc.append(c)
                nc.tensor.matmul(stat[0:1, :], ones_col_bf, c,
                                 start=(f == 0), stop=(f == FT - 1))
            for f in range(FT):
                q = big.tile([P, TC], BF16, tag="hsq", name="hsq", bufs=1)
                nc.scalar.activation(q, hc[f], AF.Square)
                nc.tensor.matmul(stat[32:33, :], ones_col_bf, q,
                                 start=(f == 0), stop=(f == FT - 1))
            s1 = small.tile([1, TC], F32, tag="s1", name="s1", bufs=1)
            nc.scalar.copy(s1, stat[0:1, :])               # mean (ones = 1/D)
            msq = small.tile([1, TC], F32, tag="msq", name="msq", bufs=1)
            nc.scalar.activation(msq, s1, AF.Square)
            s2 = small.tile([1, TC], F32, tag="s2", name="s2", bufs=1)
            nc.vector.tensor_tensor(s2, stat[32:33, :], msq, op=OP.subtract)
            nc.scalar.activation(s2, s2, AF.Ln, bias=eps_t[0:1, :], scale=1.0)
            nc.scalar.activation(s2, s2, AF.Exp, scale=-0.5)  # rstd
            return hc, s1, s2

        def rep_row(row_bf):
            """Broadcast a [1, TC] bf16 row to a [P, TC] bf16 tile via PE."""
            rp = pa.tile([P, TC], F32, tag="mm", name="rep")
            nc.tensor.matmul(rp, ones_row_bf, row_bf, start=True, stop=True)
            out = big.tile([P, TC], BF16, tag="rrep", name="rrep", bufs=4)
            nc.scalar.copy(out, rp)
            return out

        # ---------------------------------------------------------- layers
        for l in range(NL):
            w_in_t = [wpool.tile([P, 2 * DH], BF16, tag=f"w_in{k}",
                                 name=f"w_in{k}") for k in range(FT)]
            for k in range(FT):
                nc.sync.dma_start(w_in_t[k], w_in[l, k * P:(k + 1) * P, :])
            wsum_t = wpool.tile([1, 2 * DH], BF16, tag="wsum", name="wsum")
            nc.sync.dma_start(wsum_t, wsum_neg[l])
            b_in_t = [wpool.tile([P, 1], F32, tag=f"b_in{e}", name=f"b_in{e}")
                      for e in range(2 * DH // P)]
            for e in range(2 * DH // P):
                nc.sync.dma_start(b_in_t[e], b_in[l, e * P:(e + 1) * P, :])
            dcw_t = [wpool.tile([P, DCONV], F32, tag=f"dcw{g}", name=f"dcw{g}")
                     for g in range(G)]
            dcb_t = [wpool.tile([P, 1], F32, tag=f"dcb{g}", name=f"dcb{g}")
                     for g in range(G)]
            w_xp_t = [wpool.tile([P, XP], BF16, tag=f"w_xp{g}", name=f"w_xp{g}")
                      for g in range(G)]
            b_dt_t = [wpool.tile([P, 1], F32, tag=f"b_dt{g}", name=f"b_dt{g}")
                      for g in range(G)]
            ac_t = [wpool.tile([P, DS], F32, tag=f"ac{g}", name=f"ac{g}")
                    for g in range(G)]
            dpar_t = [wpool.tile([P, 1], F32, tag=f"dpar{g}", name=f"dpar{g}")
                      for g in range(G)]
            w_out_t = [wpool.tile([P, D], BF16, tag=f"w_out{g}", name=f"w_out{g}")
                       for g in range(G)]
            for g in range(G):
                s = slice(g * P, (g + 1) * P)
                nc.sync.dma_start(dcw_t[g], dconv_wt[l, s, :])
                nc.sync.dma_start(dcb_t[g], dconv_bt[l, s, :])
                nc.sync.dma_start(w_xp_t[g], w_xp[l, s, :])
                nc.sync.dma_start(b_dt_t[g], b_dt[l, s, :])
                nc.sync.dma_start(ac_t[g], a_cols[l, s, :])
                nc.sync.dma_start(dpar_t[g], d_par[l, s, :])
                nc.sync.dma_start(w_out_t[g], w_out[l, s, :])
            w_dt_t = wpool.tile([R, DH], BF16, tag="w_dt", name="w_dt")
            nc.sync.dma_start(w_dt_t, w_dt[l])

            # full-T concat buffers (xs written in-place over xraw after dconv)
            # pad to 4 so the xs view starts at an even element offset (DVE 2x)
            PAD = 4
            xraw_c = [big.tile([P, PAD + T], BF16, tag=f"xrc{g}",
                               name=f"xrc{g}") for g in range(G)]
            zs_c = [big.tile([P, T], BF16, tag=f"zsc{g}", name=f"zsc{g}")
                    for g in range(G)]
            for g in range(G):
                nc.vector.memset(xraw_c[g][:, 0:PAD], 0.0)

            # ===== phase A: stats for all chunks first (cross-chunk pipelining)
            srows = []
            for ci in range(NC):
                t0 = ci * TC
                hc, s1, s2 = ln_stats(slice(t0, t0 + TC), ci)
                rstd_bf = small.tile([1, TC], BF16, tag="rb", name="rb", bufs=1)
                nc.scalar.copy(rstd_bf, s2)
                mr_row = small.tile([1, TC], BF16, tag=f"mr{ci}", name="mr",
                                    bufs=1)
                nc.vector.tensor_tensor(mr_row, s1, s2, op=OP.mult)
                srows.append((hc, mr_row, rep_row(rstd_bf)))
            # ===== in_proj / dconv / x_proj pipelined per T-half: the hf0
            # AllReduce is issued before chunks 2-3's in_proj is emitted
            TH = T // 2
            T2 = T // 2
            xs_c = [xraw_c[g][:, PAD:PAD + T] for g in range(G)]

            def emit_inproj(ci):
                t0 = ci * TC
                sl = slice(t0, t0 + TC)
                hc, mr_row, rrep = srows[ci]
                hcn = hc
                for f in range(FT):
                    nc.vector.tensor_tensor(hcn[f], hc[f], rrep, op=OP.mult)
                for half in range(2):
                    for g in range(G):
                        e0 = half * DH + g * P
                        pt = pa.tile([P, TC], F32, tag="mm", name="mm")
                        for k in range(FT):
                            nc.tensor.matmul(pt, w_in_t[k][:, e0:e0 + P],
                                             hcn[k], start=(k == 0), stop=False)
                        nc.tensor.matmul(pt, wsum_t[:, e0:e0 + P], mr_row,
                                         start=False, stop=True)
                        bia = b_in_t[e0 // P]
                        if half == 0:
                            nc.scalar.activation(
                                xraw_c[g][:, PAD + t0:PAD + t0 + TC],
                                pt, AF.Identity, bias=bia, scale=1.0)
                        else:
                            nc.scalar.activation(zs_c[g][:, sl], pt, AF.Silu,
                                                 bias=bia, scale=1.0)

            def emit_dconv(hh):
                o0 = hh * TH
                for g in range(G):
                    tmp = small.tile([P, TH], BF16, tag="dctmp", name="dctmp",
                                     bufs=2)
                    nc.vector.tensor_scalar(tmp, xraw_c[g][:, 1 + o0:1 + o0 + TH],
                                            dcw_t[g][:, 0:1], None, OP.mult)
                    for j in range(1, DCONV):
                        tmp2 = small.tile([P, TH], BF16, tag="dctmp",
                                          name="dctmp", bufs=2)
                        nc.vector.scalar_tensor_tensor(
                            tmp2, xraw_c[g][:, 1 + j + o0:1 + j + o0 + TH],
                            dcw_t[g][:, j:j + 1], tmp, op0=OP.mult, op1=OP.add)
                        tmp = tmp2
                    nc.scalar.activation(xraw_c[g][:, PAD + o0:PAD + o0 + TH],
                                         tmp, AF.Silu, bias=dcb_t[g],
                                         scale=1.0)

            def emit_xproj(hf):
                for cj in range(2):
                    t0 = (2 * hf + cj) * TC
                    pt = pa.tile([P, TC], F32, tag="mm", name="mm")
                    for g in range(G):
                        nc.tensor.matmul(pt[0:XP, :], w_xp_t[g],
                                         xs_c[g][:, t0:t0 + TC],
                                         start=(g == 0), stop=(g == G - 1))
                    dbc_p = small.tile([XP, TC], BF16, tag="dbc_p",
                                       name="dbc_p", bufs=1)
                    nc.scalar.copy(dbc_p, pt[0:XP, :])
                    nc.sync.dma_start(
                        cc_dbc_i[2 * l + hf][:, cj * TC:(cj + 1) * TC], dbc_p)
                nc.gpsimd.collective_compute(
                    "AllReduce", OP.add, replica_groups=GROUPS,
                    ins=[cc_dbc_i[2 * l + hf][:, :]],
                    outs=[cc_dbc_o[2 * l + hf][:, :]])

            emit_inproj(0)
            emit_inproj(1)
            emit_dconv(0)
            emit_xproj(0)
            emit_inproj(2)
            emit_inproj(3)
            emit_dconv(1)
            emit_xproj(1)

            dt_c = [big.tile([P, T], BF16, tag=f"dtc{g}", name=f"dtc{g}")
                    for g in range(G)]
            dtu_c = [big.tile([P, T], BF16, tag=f"dtuc{g}", name=f"dtuc{g}")
                     for g in range(G)]
            fin = [big.tile([P, DS], F32, tag=f"fin{g}", name=f"fin{g}")
                   for g in range(G)]

            def emit_dt(hf):
                """dt = softplus(w_dt @ dbc_dt + b); dtu = dt*xs (one T-half)."""
                h0 = hf * T2
                hsl = slice(h0, h0 + T2)
                cc = cc_dbc_o[2 * l + hf]
                dbc_dt = small.tile([R, T2], BF16, tag="dbc_dt",
                                    name="dbc_dt", bufs=1)
                nc.sync.dma_start(dbc_dt, cc[0:R, :])
                # softplus: all exps first, then all lns in-place (1 table swap)
                for g in range(G):
                    for cj in range(2):
                        ptd = pb.tile([P, TC], F32, tag="mm", name="mm")
                        nc.tensor.matmul(ptd, w_dt_t[:, g * P:(g + 1) * P],
                                         dbc_dt[:, cj * TC:(cj + 1) * TC],
                                         start=True, stop=True)
                        nc.scalar.activation(
                            dt_c[g][:, h0 + cj * TC:h0 + (cj + 1) * TC], ptd,
                            AF.Exp, bias=b_dt_t[g], scale=1.0)
                for g in range(G):
                    for cj in range(2):
                        sl2 = slice(h0 + cj * TC, h0 + (cj + 1) * TC)
                        nc.scalar.activation(dt_c[g][:, sl2], dt_c[g][:, sl2],
                                             AF.Ln, bias=1.0, scale=1.0)
                for g in range(G):
                    nc.vector.tensor_tensor(dtu_c[g][:, hsl], dt_c[g][:, hsl],
                                            xs_c[g][:, hsl], op=OP.mult)

            emit_dt(0)
            for hf in range(2):
                h0 = hf * T2
                hsl = slice(h0, h0 + T2)
                cc = cc_dbc_o[2 * l + hf]
                # ---- scan blocks: g-pairs share B/C fetches
                for gp in ((0, 1), (2, 3)):
                    # overlap: emit next half's dt work mid-way so Act/PE
                    # compute it during this half's scans
                    if hf == 0 and gp == (2, 3):
                        emit_dt(1)
                    ypgs = {}
                    for g in gp:
                        yp = pyac.tile([P, T2], F32, tag=f"y{g % 2}",
                                       name=f"y{g % 2}")
                        for cj in range(2):
                            xsD = small.tile([P, TC], BF16, tag="xsD",
                                             name="xsD", bufs=1)
                            nc.vector.tensor_scalar(
                                xsD, xs_c[g][:, h0 + cj * TC:h0 + (cj + 1) * TC],
                                dpar_t[g], None, OP.mult)
                            nc.tensor.matmul(yp[:, cj * TC:(cj + 1) * TC],
                                             identb, xsD, start=True,
                                             stop=False)
                        ypgs[g] = yp

                    def fetch_q(q):
                        """One DMA per 2-row pair of B rows / C rows."""
                        bt = bc.tile([P, 2 * T2], BF16, tag=f"bq{q % 2}",
                                     name=f"bq{q % 2}")
                        ct = bc.tile([P, 2 * T2], BF16, tag=f"cq{q % 2}",
                                     name=f"cq{q % 2}")
                        bdst = bass.AP(tensor=bt.tensor, offset=bt.offset,
                                       ap=[list(bt.ap[0]), [T2, 2], [1, T2]])
                        cdst = bass.AP(tensor=ct.tensor, offset=ct.offset,
                                       ap=[list(ct.ap[0]), [T2, 2], [1, T2]])
                        nc.scalar.dma_start(
                            bdst, bcast_rows(cc[R + 2 * q:R + 2 * q + 2, :]),
                            single_packet=True)
                        nc.sync.dma_start(
                            cdst,
                            bcast_rows(cc[R + DS + 2 * q:R + DS + 2 * q + 2, :]),
                            single_packet=True)
                        return bt, ct

                    bcache = {0: fetch_q(0), 1: fetch_q(1)}
                    for q in range(8):
                        bt_q, ct_q = bcache[q % 2]
                        for ni in range(2):
                            if ni == 1 and 0 < q < 7:
                                # prefetch q+1 here so the DMA transfer
                                # overlaps ni=1's scans, not the cm/b_t ops
                                bcache[(q + 1) % 2] = fetch_q(q + 1)
                            n = 2 * q + ni
                            brep = bt_q[:, ni * T2:(ni + 1) * T2]
                            crep = ct_q[:, ni * T2:(ni + 1) * T2]
                            hscs = {}
                            for g in gp:
                                a_t = scanp.tile([P, T2], BF16, tag="sa",
                                                 name="sa", bufs=2)
                                nc.scalar.activation(a_t, dt_c[g][:, hsl],
                                                     AF.Exp,
                                                     scale=ac_t[g][:, n:n + 1])
                                b_t = scanp.tile([P, T2], BF16, tag="sb",
                                                 name="sb")
                                nc.vector.tensor_tensor(b_t, dtu_c[g][:, hsl],
                                                        brep, op=OP.mult)
                                hsc = scanp.tile([P, T2], BF16, tag="sh",
                                                 name="sh")
                                init = 0.0 if hf == 0 else fin[g][:, n:n + 1]
                                nc.vector.tensor_tensor_scan(hsc, a_t, b_t,
                                                             init,
                                                             op0=OP.mult,
                                                             op1=OP.add)
                                if hf == 0:
                                    nc.vector.tensor_copy(fin[g][:, n:n + 1],
                                                          hsc[:, T2 - 1:T2])
                                hscs[g] = hsc
                            # cm after both scans (avoid read-after-write
                            # stall on hsc); one per engine per n
                            for gi, g in enumerate(gp):
                                cm = scanp.tile([P, T2], BF16, tag="sm",
                                                name="sm")
                                nc.vector.tensor_tensor(cm, hscs[g], crep,
                                                        op=OP.mult)
                                for cj in range(2):
                                    nc.tensor.matmul(
                                        ypgs[g][:, cj * TC:(cj + 1) * TC],
                                        identb, cm[:, cj * TC:(cj + 1) * TC],
                                        start=False, stop=(n == DS - 1))
                    # gating into dtu_c (dead after n loop)
                    for g in gp:
                        nc.vector.tensor_tensor(dtu_c[g][:, hsl], ypgs[g],
                                                zs_c[g][:, hsl], op=OP.mult)

                # ---- out_proj + AR_y + residual for this half's chunks
                for cj in range(2):
                    ci = 2 * hf + cj
                    t0 = ci * TC
                    sl = slice(t0, t0 + TC)
                    ccidx = l * NC + ci
                    for f in range(FT):
                        po = pb.tile([P, TC], F32, tag="mm", name="mm")
                        for g in range(G):
                            nc.tensor.matmul(po,
                                             w_out_t[g][:, f * P:(f + 1) * P],
                                             dtu_c[g][:, sl], start=(g == 0),
                                             stop=(g == G - 1))
                        ot = small.tile([P, TC], BF16, tag="oout", name="oout")
                        nc.scalar.copy(ot, po)
                        nc.sync.dma_start(cc_y_i[ccidx][f * P:(f + 1) * P, :],
                                          ot)
                    nc.gpsimd.collective_compute(
                        "AllReduce", OP.add, replica_groups=GROUPS,
                        ins=[cc_y_i[ccidx][:, :]], outs=[cc_y_o[ccidx][:, :]])
                    for f in range(FT):
                        yfull = scanp.tile([P, TC], BF16, tag="yfull",
                                           name="yfull", bufs=2)
                        nc.sync.dma_start(yfull,
                                          cc_y_o[ccidx][f * P:(f + 1) * P, :])
                        nc.vector.tensor_tensor(h[f][:, sl], h[f][:, sl],
                                                yfull, op=OP.add)

        # ------------------------------------------- final LN + transpose out
        fn_t = [persist.tile([P, 2], F32, name=f"fn{f}") for f in range(FT)]
        for f in range(FT):
            nc.sync.dma_start(fn_t[f], fn_wb[f * P:(f + 1) * P, :])
        for ci in range(NC):
            t0 = ci * TC
            sl = slice(t0, t0 + TC)
            hc, s1, s2 = ln_stats(sl, ci)
            s1_bf = small.tile([1, TC], BF16, tag=f"mr{ci}", name="mr", bufs=1)
            nc.vector.tensor_copy(s1_bf, s1)
            rstd_bf = small.tile([1, TC], BF16, tag=f"rb{ci}", name="rb",
                                 bufs=1)
            nc.vector.tensor_copy(rstd_bf, s2)
            mrep = rep_row(s1_bf)
            rrep = rep_row(rstd_bf)
            hn = []
            for f in range(FT):
                t2 = big.tile([P, TC], F32, tag=f"fhn{f}", name=f"hn{f}")
                nc.vector.tensor_tensor(t2, hc[f], mrep, op=OP.subtract)
                nc.vector.tensor_tensor(t2, t2, rrep, op=OP.mult)
                nc.vector.tensor_scalar(t2, t2, fn_t[f][:, 0:1],
                                        fn_t[f][:, 1:2], OP.mult, OP.add)
                hn.append(t2)
            for tb in range(TC // P):
                ht = small.tile([P, D], BF16, tag="ht", name="ht", bufs=1)
                for f in range(FT):
                    pt = pa.tile([P, TC], F32, tag="mm", name="mm")
                    nc.tensor.transpose(pt[:, 0:P], hn[f][:, tb * P:(tb + 1) * P],
                                        identf)
                    nc.scalar.copy(ht[:, f * P:(f + 1) * P], pt[:, 0:P])
                tglob = t0 + tb * P
                src = ht[:, :]
                rep_in = bass.AP(tensor=src.tensor, offset=src.offset,
                                 ap=[list(src.ap[0]), [0, STRIDE],
                                     list(src.ap[1])])
                dst = y_out[STRIDE * tglob:STRIDE * (tglob + P), :]
                dst3 = dst.rearrange("(t r) d -> t r d", r=STRIDE)
                nc.sync.dma_start(dst3, rep_in)

    nc.compile()
    return nc


# ================================================================ host side
def make_core_inputs(inputs, T=2048, NL=4):
    x = np.asarray(inputs["x"], np.float32)
    conv_w = np.asarray(inputs["conv_w"], np.float32)
    conv_b = np.asarray(inputs["conv_b"], np.float32)
    in_proj_w = np.asarray(inputs["in_proj_w"], np.float32)
    dconv_w = np.asarray(inputs["dconv_w"], np.float32)
    dconv_b = np.asarray(inputs["dconv_b"], np.float32)
    x_proj_w = np.asarray(inputs["x_proj_w"], np.float32)
    dt_proj_w = np.asarray(inputs["dt_proj_w"], np.float32)
    dt_proj_b = np.asarray(inputs["dt_proj_b"], np.float32)
    A_log = np.asarray(inputs["A_log"], np.float32)
    D_param = np.asarray(inputs["D_param"], np.float32)
    out_proj_w = np.asarray(inputs["out_proj_w"], np.float32)
    ln_w = np.asarray(inputs["ln_w"], np.float32)
    ln_b = np.asarray(inputs["ln_b"], np.float32)
    fn_w = np.asarray(inputs["fn_w"], np.float32)
    fn_b = np.asarray(inputs["fn_b"], np.float32)

    Bn = x.shape[0]
    di = x.shape[2]
    dmodel = conv_w.shape[0]
    dinner = in_proj_w.shape[1] // 2
    dh = dinner // 2

    xpad = np.concatenate([np.zeros((Bn, KF - 1, di), np.float32), x], axis=1)
    idx = np.arange(T)[:, None] * STRIDE + np.arange(KF)[None, :]
    xcat = xpad[:, idx, :].reshape(Bn, T, KF * di)
    xcatT = np.ascontiguousarray(xcat.transpose(0, 2, 1))
    wconv = np.ascontiguousarray(conv_w.transpose(2, 1, 0).reshape(KF * di, dmodel))

    A = -np.exp(A_log)

    per_core = []
    for c in range(8):
        b, j = c // 2, c % 2
        sl = slice(j * dh, (j + 1) * dh)
        w_in_l, b_in_l, w_out_l, w_xp_l, wsum_l = [], [], [], [], []
        for l in range(NL):
            Wx = in_proj_w[l, :dinner][sl] * ln_w[l][None, :]
            Wz = in_proj_w[l, dinner:][sl] * ln_w[l][None, :]
            wl = np.concatenate([Wx.T, Wz.T], axis=1)
            w_in_l.append(wl)
            wsum_l.append(-wl.sum(axis=0, keepdims=True))
            bx = in_proj_w[l, :dinner][sl] @ ln_b[l]
            bz = in_proj_w[l, dinner:][sl] @ ln_b[l]
            b_in_l.append(np.concatenate([bx, bz])[:, None])
            w_out_l.append(out_proj_w[l][:, sl].T)
            w_xp_l.append(np.ascontiguousarray(x_proj_w[l][:, sl].T))
        d = dict(
            xcatT=xcatT[b],
            wconv=wconv,
            conv_bias=conv_b[:, None],
            w_in=np.stack(w_in_l),
            b_in=np.stack(b_in_l),
            wsum_neg=np.stack(wsum_l),
            dconv_wt=dconv_w[:, sl, :],
            dconv_bt=dconv_b[:, sl, None],
            w_xp=np.stack(w_xp_l),
            w_dt=np.ascontiguousarray(dt_proj_w[:, sl, :].transpose(0, 2, 1)),
            b_dt=dt_proj_b[:, sl, None],
            a_cols=A[:, sl, :],
            d_par=D_param[:, sl, None],
            w_out=np.stack(w_out_l),
            fn_wb=np.stack([fn_w, fn_b], axis=1),
            identb_bf=np.eye(P, dtype=np.float32),
        )
        per_core.append(d)
    return per_core


def cast_core_inputs(nc, per_core):
    import concourse.mybir as mybir
    want = {}
    for alloc in nc.m.functions[0].allocations:
        if getattr(alloc, "kind", None) == "ExternalInput":
            want[alloc.memorylocations[0].name] = mybir.dt.np(alloc.dtype)
    return [{k: np.ascontiguousarray(np.asarray(v).astype(want[k]))
             for k, v in d.items() if k in want} for d in per_core]


_PROGRAM_CACHE = {}


def get_program(T=2048, NL=4, TC=512):
    key = (T, NL, TC)
    if key not in _PROGRAM_CACHE:
        _PROGRAM_CACHE[key] = build_program(T, NL, TC)
    return _PROGRAM_CACHE[key]


def kernel(**inputs):
    from concourse.bass_utils import run_bass_kernel_spmd
    T = inputs["x"].shape[1] // STRIDE
    NL = inputs["in_proj_w"].shape[0]
    nc = get_program(T, NL)
    per_core = cast_core_inputs(nc, make_core_inputs(inputs, T, NL))
    res = run_bass_kernel_spmd(nc, per_core, core_ids=list(range(8)))
    Bn = inputs["x"].shape[0]
    y = np.stack([res.results[2 * b]["y_out"] for b in range(Bn)])
    return y.astype(np.float32)

